# revision 3
# baseline (speedup 1.0000x reference)
"""Trainium2 Bass kernel for nn_MultiHeadAttention_48395691492101.

Strategy: pure head-parallel sharding across 8 NeuronCores (2 heads/core).
Because the reference reshapes ctx [B,H,T,DV] -> [B,T,H*DV] WITHOUT
transposing, row-block t' in [h*128,(h+1)*128) of the reshaped tensor comes
entirely from head h.  Core c (heads 2c,2c+1) therefore owns output rows
[c*256,(c+1)*256) of every batch, and the output projection needs no
cross-core reduction at all.

Datapath (v2): bf16 on every matmul input (PSUM accumulation stays fp32),
two heads fused into single [128,1024] score/exp/context tiles, causal
handled by chunk skipping + a triangular -8e9 add on diagonal blocks +
column-restricted exp/PV (dead columns are never touched), per-q-group
softmax normalization (DVE reciprocal of the matmul-produced row sums +
gpsimd partition_broadcast) folded into the Y^T gather copies, and bulk
DMAs kept off the Activation queue so exp dispatch never stalls.
"""

import sys

if "/opt/trn_rl_repo" not in sys.path:
    sys.path.insert(0, "/opt/trn_rl_repo")

import numpy as np
import ml_dtypes

BF16 = ml_dtypes.bfloat16

B, T, D = 4, 2048, 1024
H, DK, DV = 16, 64, 64
SCALE = np.float32(1.0 / 8.0)
NCORES = 8
HP = H // NCORES          # heads per core = 2
ROWS = HP * (T * DV) // D  # output rows per head pair per batch = 256
NDC = D // 128            # 8 d-chunks
NTG = 4                   # t-groups of 512 for QKV
NQG = 4                   # q-groups of 512
NKC = T // 128            # 16 k-chunks
MASK_NEG = np.float32(-8.0e9)   # becomes -1e9 after *SCALE inside exp

_cache = {}


def _build(causal: bool, debug: bool = False):
    import concourse.tile as tile
    import concourse.mybir as mybir
    from concourse import bacc

    F32 = mybir.dt.float32
    F32R = mybir.dt.float32r
    BF = mybir.dt.bfloat16
    Exp = mybir.ActivationFunctionType.Exp

    nc = bacc.Bacc("TRN2", target_bir_lowering=False, debug=False,
                   num_devices=NCORES)

    # host-prearranged layouts: xT [128, dc, B*T], w* [128, dc*128+j]
    xT_d = nc.dram_tensor("xT", [128, NDC, B * T], BF, kind="ExternalInput").ap()
    wq_d = nc.dram_tensor("wq", [128, D], BF, kind="ExternalInput").ap()
    wk_d = nc.dram_tensor("wk", [128, D], BF, kind="ExternalInput").ap()
    wv_d = nc.dram_tensor("wv", [128, D], BF, kind="ExternalInput").ap()
    bq_d = nc.dram_tensor("bq", [128, 1], F32, kind="ExternalInput").ap()
    bk_d = nc.dram_tensor("bk", [128, 1], F32, kind="ExternalInput").ap()
    bv_d = nc.dram_tensor("bv", [128, 1], F32, kind="ExternalInput").ap()
    wout_d = nc.dram_tensor("wout", [128, NDC * D], BF, kind="ExternalInput").ap()
    bout_d = nc.dram_tensor("bout", [1, D], F32R, kind="ExternalInput").ap()
    onesr_d = nc.dram_tensor("onesr", [1, 128], F32R, kind="ExternalInput").ap()
    drop_d = nc.dram_tensor("drop", [B, ROWS, D], BF, kind="ExternalInput").ap()
    id_d = nc.dram_tensor("idm", [128, 128], BF, kind="ExternalInput").ap()
    if causal:
        dmask_d = nc.dram_tensor("dmask", [128, 256], F32,
                                 kind="ExternalInput").ap()
    else:
        maskT_d = nc.dram_tensor("maskT", [T, T], F32, kind="ExternalInput").ap()
    out_d = nc.dram_tensor("out", [B, ROWS, D], F32, kind="ExternalOutput").ap()

    with tile.TileContext(nc) as tc:
        with tc.tile_pool(name="const", bufs=1) as cpool, \
             tc.tile_pool(name="perb", bufs=1) as perb, \
             tc.tile_pool(name="stream", bufs=3) as stream, \
             tc.tile_pool(name="psum", bufs=1, space="PSUM") as pp:

            # ---- constants ----
            # gpsimd queue: QKV weights + small tensors (needed first)
            wq_sb = cpool.tile([128, D], BF)
            wk_sb = cpool.tile([128, D], BF)
            wv_sb = cpool.tile([128, D], BF)
            nc.gpsimd.dma_start(wq_sb[:], wq_d[:])
            nc.gpsimd.dma_start(wk_sb[:], wk_d[:])
            nc.gpsimd.dma_start(wv_sb[:], wv_d[:])
            bq_sb = cpool.tile([128, 1], F32)
            bk_sb = cpool.tile([128, 1], F32)
            bv_sb = cpool.tile([128, 1], F32)
            nc.gpsimd.dma_start(bq_sb[:], bq_d[:])
            nc.gpsimd.dma_start(bk_sb[:], bk_d[:])
            nc.gpsimd.dma_start(bv_sb[:], bv_d[:])
            id_sb = cpool.tile([128, 128], BF)
            nc.gpsimd.dma_start(id_sb[:], id_d[:])
            if causal:
                dmask_sb = cpool.tile([128, 256], F32)
                nc.gpsimd.dma_start(dmask_sb[:], dmask_d[:])
            # scalar queue: output-projection constants (needed much later)
            wout_sb = cpool.tile([128, NDC * D], BF)
            nc.scalar.dma_start(wout_sb[:], wout_d[:])
            bout_sb = cpool.tile([1, D], F32R)
            nc.scalar.dma_start(bout_sb[:], bout_d[:])
            ones_row = cpool.tile([1, 128], F32R)
            nc.scalar.dma_start(ones_row[:], onesr_d[:])

            for b in range(B):
                # ---------- phase 1: QKV projections ----------
                qt = perb.tile([128, T], BF, bufs=2)
                kt = perb.tile([128, T], BF, bufs=2)
                vt = perb.tile([128, T], BF, bufs=2)
                for tg in range(NTG):
                    xt = stream.tile([128, NDC * 512], BF, tag="xt", bufs=4)
                    c0 = b * T + tg * 512
                    nc.sync.dma_start(
                        xt.rearrange("p (dc j) -> p dc j", j=512),
                        xT_d[:, :, c0:c0 + 512])
                    for w_sb, bias_sb, dst in ((wq_sb, bq_sb, qt),
                                               (wk_sb, bk_sb, kt),
                                               (wv_sb, bv_sb, vt)):
                        ps = pp.tile([128, 512], F32, tag="mm", bufs=2)
                        for dc in range(NDC):
                            nc.tensor.matmul(
                                ps[:], w_sb[:, dc * 128:(dc + 1) * 128],
                                xt[:, dc * 512:(dc + 1) * 512],
                                start=(dc == 0), stop=(dc == NDC - 1))
                        nc.vector.tensor_scalar_add(
                            dst[:, tg * 512:(tg + 1) * 512], ps[:], bias_sb[:])

                # ---------- phase 2: V transpose (vnb: [kc][two][64v+1one+1pad]) ----------
                vnb = perb.tile([128, NKC * 132], BF, bufs=2)
                nc.vector.memset(
                    vnb.rearrange("p (c two w) -> p c two w", two=2, w=66)
                    [:, :, :, 64:65], 1.0)
                for kc in range(NKC):
                    tp = pp.tile([128, 128], BF, tag="mm", bufs=2)
                    nc.tensor.transpose(tp[:], vt[:, kc * 128:(kc + 1) * 128],
                                        id_sb[:])
                    nc.vector.tensor_copy(
                        vnb.rearrange("p (c two w) -> p c two w", two=2, w=66)
                        [:, kc, :, 0:64],
                        tp[:].rearrange("p (two v) -> p two v", two=2))

                # ---------- phase 3: attention (both heads fused) ----------
                ex = perb.tile([65, 2 * T], F32, bufs=1)     # [65, h*T + q]
                bcast = perb.tile([64, 2 * T], F32, bufs=1)  # recip(sums) bcast
                for qg in range(NQG):
                    kcmax = 4 * qg + 4 if causal else NKC
                    cs = pp.tile([65, 1024], F32, tag="cs", bufs=1)
                    for kc in range(kcmax):
                        o = kc - 4 * qg
                        diag = causal and o >= 0
                        live = o * 128 if diag else 0
                        st = pp.tile([128, 1024], F32, tag="st", bufs=2)
                        for h in range(2):
                            nc.tensor.matmul(
                                st[:, h * 512 + live:(h + 1) * 512],
                                kt[64 * h:64 * h + 64,
                                   kc * 128:(kc + 1) * 128],
                                qt[64 * h:64 * h + 64,
                                   qg * 512 + live:(qg + 1) * 512],
                                start=True, stop=True)
                        if diag:
                            sdv = st.rearrange("p (two n) -> p two n", two=2)
                            nc.vector.tensor_add(
                                sdv[:, :, live:live + 128],
                                sdv[:, :, live:live + 128],
                                dmask_sb.rearrange("p (two n) -> p two n",
                                                   two=2))
                        elif not causal:
                            mt = stream.tile([128, 512], F32, tag="mt", bufs=3)
                            nc.sync.dma_start(
                                mt[:],
                                maskT_d[kc * 128:(kc + 1) * 128,
                                        qg * 512:(qg + 1) * 512])
                            nc.vector.tensor_add(st[:, 0:512], st[:, 0:512],
                                                 mt[:])
                            nc.vector.tensor_add(st[:, 512:1024],
                                                 st[:, 512:1024], mt[:])
                        at = stream.tile([128, 1024], BF, tag="at", bufs=4)
                        nc.scalar.activation(
                            at.rearrange("p (two n) -> p two n", two=2)
                            [:, :, live:512],
                            st.rearrange("p (two n) -> p two n", two=2)
                            [:, :, live:512],
                            Exp, scale=float(SCALE))
                        for h in range(2):
                            nc.tensor.matmul(
                                cs[:, h * 512 + live:(h + 1) * 512],
                                vnb[:, kc * 132 + 66 * h:
                                    kc * 132 + 66 * h + 65],
                                at[:, h * 512 + live:(h + 1) * 512],
                                start=(kc == 0), stop=(kc == kcmax - 1),
                                skip_group_check=True)
                    # evict + per-qg normalization setup
                    nc.vector.tensor_copy(
                        ex.rearrange("p (two t) -> p two t", two=2)
                        [:, :, qg * 512:(qg + 1) * 512],
                        cs.rearrange("p (two n) -> p two n", two=2))
                    for h in range(2):
                        s0 = h * T + qg * 512
                        rcp = stream.tile([1, 512], F32, tag="rcp", bufs=4)
                        nc.vector.reciprocal(rcp[:], ex[64:65, s0:s0 + 512])
                        nc.gpsimd.partition_broadcast(
                            bcast[:, s0:s0 + 512], rcp[:])

                # ---------- phase 5: normalize-gather + output projection ----------
                for h in range(HP):
                    cvu = ex[0:64, h * T:(h + 1) * T].rearrange(
                        "p (r s2 two) -> p two s2 r", two=2, s2=8)
                    bcu = bcast[:, h * T:(h + 1) * T].rearrange(
                        "p (r s2 two) -> p two s2 r", two=2, s2=8)
                    yts = stream.tile([128, NDC * 128], BF, tag="yt", bufs=2)
                    ytv = yts.rearrange("p (c r) -> p c r", r=128)
                    nc.vector.tensor_mul(ytv[0:64, :, :], cvu[:, 0, :, :],
                                         bcu[:, 0, :, :])
                    nc.vector.tensor_mul(ytv[64:128, :, :], cvu[:, 1, :, :],
                                         bcu[:, 1, :, :])
                    dt2 = stream.tile([128, D], BF, tag="dt", bufs=2)
                    nc.sync.dma_start(
                        dt2[:], drop_d[b, h * 128:(h + 1) * 128, :])
                    ost = stream.tile([128, D], F32, tag="ost", bufs=2)
                    for og in range(2):
                        po = pp.tile([128, 512], F32, tag="mm", bufs=2)
                        for cc in range(NDC):
                            nc.tensor.matmul(
                                po[:], yts[:, cc * 128:(cc + 1) * 128],
                                wout_sb[:, cc * D + og * 512:
                                        cc * D + og * 512 + 512],
                                start=(cc == 0), stop=False)
                        nc.tensor.matmul(po[:], ones_row[:],
                                         bout_sb[0:1, og * 512:(og + 1) * 512],
                                         start=False, stop=True)
                        nc.vector.tensor_mul(
                            ost[:, og * 512:(og + 1) * 512], po[:],
                            dt2[:, og * 512:(og + 1) * 512])
                    nc.gpsimd.dma_start(
                        out_d[b, h * 128:(h + 1) * 128, :], ost[:])

    nc.compile()
    return nc


def _get_program(causal: bool):
    key = ("causal" if causal else "full")
    if key not in _cache:
        _cache[key] = _build(causal)
    return _cache[key]


def _host_fallback(x, attn_mask, Wq, bq, Wk, bk, Wv, bv, Wout, bout,
                   dropout_mask):
    x64 = x.astype(np.float32)
    Q = np.einsum("btd,hdk->bhtk", x64, Wq) + bq[None, :, None, :]
    K = np.einsum("btd,hdk->bhtk", x64, Wk) + bk[None, :, None, :]
    V = np.einsum("btd,hdv->bhtv", x64, Wv) + bv[None, :, None, :]
    scores = np.einsum("bhqk,bhmk->bhqm", Q, K) * SCALE + attn_mask
    scores = scores - scores.max(-1, keepdims=True)
    e = np.exp(scores)
    attn = e / e.sum(-1, keepdims=True)
    ctx = np.einsum("bhqm,bhmv->bhqv", attn, V).reshape(B, T, H * DV)
    out = ctx @ Wout.T + bout
    return (out * dropout_mask).astype(np.float32)


def _chunked128(w):
    """[D, M] -> [128, (dc M)] with row d = dc*128 + p."""
    d, m = w.shape
    return np.ascontiguousarray(
        w.reshape(d // 128, 128, m).transpose(1, 0, 2).reshape(128, -1))


def kernel(x, attn_mask, Wq, bq, Wk, bk, Wv, bv, Wout, bout, dropout_mask):
    from concourse.bass_utils import run_bass_kernel_spmd

    x = np.ascontiguousarray(x, np.float32)
    m = np.asarray(attn_mask, np.float32).reshape(T, T)

    # causality check on the actual mask tensor
    causal = bool((np.tril(m) == 0).all() and
                  (m[np.triu_indices(T, 1)] <= -1e8).all())

    # safety: cheap bound on max |scaled score| -> exp overflow guard
    xf = x.reshape(B * T, D)
    Qa = xf @ Wq.transpose(1, 0, 2).reshape(D, H * DK)
    Ka = xf @ Wk.transpose(1, 0, 2).reshape(D, H * DK)
    Qa = Qa.reshape(B * T, H, DK) + bq[None]
    Ka = Ka.reshape(B * T, H, DK) + bk[None]
    qn = np.linalg.norm(Qa, axis=2).max(0)     # per-head max row norm
    kn = np.linalg.norm(Ka, axis=2).max(0)
    bound = float(SCALE) * float((qn * kn).max())
    if bound > 50.0:
        return _host_fallback(x, attn_mask, Wq, bq, Wk, bk, Wv, bv, Wout,
                              bout, dropout_mask)

    nc = _get_program(causal)

    # xT [128, dc, B*T] bf16
    xT = np.ascontiguousarray(
        x.transpose(2, 0, 1).reshape(NDC, 128, B * T).transpose(1, 0, 2)
    ).astype(BF)
    woutT = np.asarray(Wout, np.float32).T            # [f, o]
    wout_sb = _chunked128(woutT).astype(BF)
    boutr = np.asarray(bout, np.float32).reshape(1, D)
    idm = np.eye(128, dtype=np.float32).astype(BF)
    dmask1 = np.where(np.arange(128)[None, :] < np.arange(128)[:, None],
                      MASK_NEG, np.float32(0.0)).astype(np.float32)
    dmask = np.concatenate([dmask1, dmask1], axis=1)
    maskT = None if causal else np.ascontiguousarray(m.T * np.float32(8.0))
    drop = np.asarray(dropout_mask, np.float32).astype(BF)

    in_maps = []
    for c in range(NCORES):
        h0, h1 = HP * c, HP * c + 1
        im = {
            "xT": xT,
            "wq": _chunked128(
                np.concatenate([Wq[h0], Wq[h1]], axis=1)).astype(BF),
            "wk": _chunked128(
                np.concatenate([Wk[h0], Wk[h1]], axis=1)).astype(BF),
            "wv": _chunked128(
                np.concatenate([Wv[h0], Wv[h1]], axis=1)).astype(BF),
            "bq": np.concatenate([bq[h0], bq[h1]]).reshape(128, 1)
                    .astype(np.float32),
            "bk": np.concatenate([bk[h0], bk[h1]]).reshape(128, 1)
                    .astype(np.float32),
            "bv": np.concatenate([bv[h0], bv[h1]]).reshape(128, 1)
                    .astype(np.float32),
            "wout": wout_sb,
            "bout": boutr,
            "onesr": np.ones((1, 128), np.float32),
            "drop": np.ascontiguousarray(drop[:, c * ROWS:(c + 1) * ROWS, :]),
            "idm": idm,
        }
        if causal:
            im["dmask"] = dmask
        else:
            im["maskT"] = maskT
        in_maps.append(im)

    res = run_bass_kernel_spmd(nc, in_maps, list(range(NCORES)))
    out = np.empty((B, T, D), np.float32)
    for c in range(NCORES):
        out[:, c * ROWS:(c + 1) * ROWS, :] = res.results[c]["out"]
    return out


# revision 4
# speedup vs baseline: 1.1849x; 1.1849x over previous
"""Trainium2 Bass kernel for nn_MultiHeadAttention_48395691492101.

Strategy: pure head-parallel sharding across 8 NeuronCores (2 heads/core).
Because the reference reshapes ctx [B,H,T,DV] -> [B,T,H*DV] WITHOUT
transposing, row-block t' in [h*128,(h+1)*128) of the reshaped tensor comes
entirely from head h.  Core c (heads 2c,2c+1) therefore owns output rows
[c*256,(c+1)*256) of every batch, and the output projection needs no
cross-core reduction at all.

v3 datapath:
- matmul *moving* operands in bf16 (cost: 1 cycle/row at any width, halves
  the x DMA), *stationary* operands in f32r (self-loading matmuls - no
  Ldweights instructions on the PE sequencer).
- two heads fused per tile: scores/exp tiles are [128, 1024], the PV
  accumulator is [65, 1024] (64 V rows + a ones row that yields softmax
  denominators for free).
- causal: dead 128x512 blocks skipped, diagonal blocks get a triangular
  -8e9 add and column-restricted exp/PV (dead columns never touched).
- per-q-group normalization: one [65,2,512] PSUM->SBUF evict, DVE
  reciprocal of the sums row, gpsimd partition_broadcast; the multiply is
  folded into the Y^T gather copies of the output projection.
- software pipelining: scores are emitted one k-chunk ahead of PV so the
  PE never blocks on exp; QKV of batch b+1 is interleaved at q-group
  boundaries of batch b's attention (covers the Act-bound stretches and
  the PSUM accumulator evict); output stores are emitted one batch late so
  their semaphore waits never block the Pool queue.
"""

import sys

if "/opt/trn_rl_repo" not in sys.path:
    sys.path.insert(0, "/opt/trn_rl_repo")

import numpy as np
import ml_dtypes

BF16 = ml_dtypes.bfloat16

B, T, D = 4, 2048, 1024
H, DK, DV = 16, 64, 64
SCALE = np.float32(1.0 / 8.0)
NCORES = 8
HP = H // NCORES          # heads per core = 2
ROWS = HP * (T * DV) // D  # output rows per head pair per batch = 256
NDC = D // 128            # 8 d-chunks
NTG = 4                   # t-groups of 512 for QKV
NQG = 4                   # q-groups of 512
NKC = T // 128            # 16 k-chunks
MASK_NEG = np.float32(-8.0e9)   # becomes -1e9 after *SCALE inside exp

_cache = {}


def _build(causal: bool, debug: bool = False):
    import concourse.tile as tile
    import concourse.mybir as mybir
    from concourse import bacc

    F32 = mybir.dt.float32
    F32R = mybir.dt.float32r
    BF = mybir.dt.bfloat16
    Exp = mybir.ActivationFunctionType.Exp

    nc = bacc.Bacc("TRN2", target_bir_lowering=False, debug=False,
                   num_devices=NCORES)

    # host-prearranged layouts: xT [128, dc, B*T], w* [128, dc*128+j]
    xT_d = nc.dram_tensor("xT", [128, NDC, B * T], BF, kind="ExternalInput").ap()
    wq_d = nc.dram_tensor("wq", [128, D], F32R, kind="ExternalInput").ap()
    wk_d = nc.dram_tensor("wk", [128, D], F32R, kind="ExternalInput").ap()
    wv_d = nc.dram_tensor("wv", [128, D], F32R, kind="ExternalInput").ap()
    bq_d = nc.dram_tensor("bq", [128, 1], F32, kind="ExternalInput").ap()
    bk_d = nc.dram_tensor("bk", [128, 1], F32, kind="ExternalInput").ap()
    bv_d = nc.dram_tensor("bv", [128, 1], F32, kind="ExternalInput").ap()
    wout_d = nc.dram_tensor("wout", [128, NDC * D], BF, kind="ExternalInput").ap()
    bout_d = nc.dram_tensor("bout", [1, D], F32R, kind="ExternalInput").ap()
    onesr_d = nc.dram_tensor("onesr", [1, 128], F32R, kind="ExternalInput").ap()
    drop_d = nc.dram_tensor("drop", [B, ROWS, D], BF, kind="ExternalInput").ap()
    id_d = nc.dram_tensor("idm", [128, 128], BF, kind="ExternalInput").ap()
    if causal:
        dmask_d = nc.dram_tensor("dmask", [128, 256], F32,
                                 kind="ExternalInput").ap()
    else:
        maskT_d = nc.dram_tensor("maskT", [T, T], F32, kind="ExternalInput").ap()
    out_d = nc.dram_tensor("out", [B, ROWS, D], F32, kind="ExternalOutput").ap()

    with tile.TileContext(nc) as tc:
        with tc.tile_pool(name="const", bufs=1) as cpool, \
             tc.tile_pool(name="perb", bufs=1) as perb, \
             tc.tile_pool(name="stream", bufs=3) as stream, \
             tc.tile_pool(name="psum", bufs=1, space="PSUM") as pp:

            # ---- constants ----
            # gpsimd queue: QKV weights + small tensors (needed first)
            wq_sb = cpool.tile([128, D], F32R, tag="wq")
            wk_sb = cpool.tile([128, D], F32R, tag="wk")
            wv_sb = cpool.tile([128, D], F32R, tag="wv")
            nc.gpsimd.dma_start(wq_sb[:], wq_d[:])
            nc.gpsimd.dma_start(wk_sb[:], wk_d[:])
            nc.gpsimd.dma_start(wv_sb[:], wv_d[:])
            bq_sb = cpool.tile([128, 1], F32, tag="bq")
            bk_sb = cpool.tile([128, 1], F32, tag="bk")
            bv_sb = cpool.tile([128, 1], F32, tag="bv")
            nc.gpsimd.dma_start(bq_sb[:], bq_d[:])
            nc.gpsimd.dma_start(bk_sb[:], bk_d[:])
            nc.gpsimd.dma_start(bv_sb[:], bv_d[:])
            id_sb = cpool.tile([128, 128], BF, tag="idm")
            nc.gpsimd.dma_start(id_sb[:], id_d[:])
            if causal:
                dmask_sb = cpool.tile([128, 256], F32, tag="dmask")
                nc.gpsimd.dma_start(dmask_sb[:], dmask_d[:])
            # scalar queue: output-projection constants (needed much later)
            wout_sb = cpool.tile([128, NDC * D], BF, tag="wout")
            nc.scalar.dma_start(wout_sb[:], wout_d[:])
            bout_sb = cpool.tile([1, D], F32R, tag="bout")
            nc.scalar.dma_start(bout_sb[:], bout_d[:])
            ones_row = cpool.tile([1, 128], F32R, tag="onesr")
            nc.scalar.dma_start(ones_row[:], onesr_d[:])

            def alloc_qkv():
                qt = perb.tile([128, T], BF, tag="qt", bufs=2, name="qt")
                kt = perb.tile([128, T], F32R, tag="kt", bufs=2, name="kt")
                vt = perb.tile([128, T], F32R, tag="vt", bufs=1, name="vt")
                return qt, kt, vt

            def emit_qkv_tg(qkv, b, tg):
                qt, kt, vt = qkv
                xt = stream.tile([128, NDC * 512], BF, tag="xt", bufs=3,
                                 name="xt")
                c0 = b * T + tg * 512
                nc.sync.dma_start(
                    xt.rearrange("p (dc j) -> p dc j", j=512),
                    xT_d[:, :, c0:c0 + 512])
                for w_sb, bias_sb, dst in ((wq_sb, bq_sb, qt),
                                           (wk_sb, bk_sb, kt),
                                           (wv_sb, bv_sb, vt)):
                    ps = pp.tile([128, 512], F32, tag="mm", bufs=2, name="ps")
                    for dc in range(NDC):
                        nc.tensor.matmul(
                            ps[:], w_sb[:, dc * 128:(dc + 1) * 128],
                            xt[:, dc * 512:(dc + 1) * 512],
                            start=(dc == 0), stop=(dc == NDC - 1))
                    nc.vector.tensor_scalar_add(
                        dst[:, tg * 512:(tg + 1) * 512], ps[:], bias_sb[:])

            def emit_transposes(vt):
                # vnb layout per k-chunk: [two heads][64 V rows + ones + pad]
                vnb = perb.tile([128, NKC * 132], F32R, tag="vnb", bufs=2,
                                name="vnb")
                nc.vector.memset(
                    vnb.rearrange("p (c two w) -> p c two w", two=2, w=66)
                    [:, :, :, 64:65], 1.0)
                for kc in range(NKC):
                    tp = pp.tile([128, 128], F32R, tag="mm", bufs=2, name="tp")
                    nc.tensor.transpose(tp[:], vt[:, kc * 128:(kc + 1) * 128],
                                        id_sb[:])
                    nc.vector.tensor_copy(
                        vnb.rearrange("p (c two w) -> p c two w", two=2, w=66)
                        [:, kc, :, 0:64],
                        tp[:].rearrange("p (two v) -> p two v", two=2))
                return vnb

            def emit_scores(qkv, qg, kc, live, diag):
                qt, kt, vt = qkv
                st = pp.tile([128, 1024], F32, tag="st", bufs=2, name="st")
                for h in range(2):
                    nc.tensor.matmul(
                        st[:, h * 512 + live:(h + 1) * 512],
                        kt[64 * h:64 * h + 64, kc * 128:(kc + 1) * 128],
                        qt[64 * h:64 * h + 64,
                           qg * 512 + live:(qg + 1) * 512],
                        start=True, stop=True)
                if diag:
                    sdv = st.rearrange("p (two n) -> p two n", two=2)
                    nc.vector.tensor_add(
                        sdv[:, :, live:live + 128],
                        sdv[:, :, live:live + 128],
                        dmask_sb.rearrange("p (two n) -> p two n", two=2))
                elif not causal:
                    mt = stream.tile([128, 512], F32, tag="mt", bufs=3,
                                     name="mt")
                    nc.sync.dma_start(
                        mt[:], maskT_d[kc * 128:(kc + 1) * 128,
                                       qg * 512:(qg + 1) * 512])
                    nc.vector.tensor_add(st[:, 0:512], st[:, 0:512], mt[:])
                    nc.vector.tensor_add(st[:, 512:1024], st[:, 512:1024],
                                         mt[:])
                at = stream.tile([128, 1024], BF, tag="at", bufs=4, name="at")
                nc.scalar.activation(
                    at.rearrange("p (two n) -> p two n", two=2)[:, :, live:512],
                    st.rearrange("p (two n) -> p two n", two=2)[:, :, live:512],
                    Exp, scale=float(SCALE))
                return at

            def emit_pv(cs, vnb, at, kc, live, first, last):
                for h in range(2):
                    nc.tensor.matmul(
                        cs[:, h * 512 + live:(h + 1) * 512],
                        vnb[:, kc * 132 + 66 * h:kc * 132 + 66 * h + 65],
                        at[:, h * 512 + live:(h + 1) * 512],
                        start=first, stop=last, skip_group_check=True)

            def emit_attn_qg(b, qg, qkv, vnb, ex, bcast):
                kcmax = 4 * qg + 4 if causal else NKC
                cs = pp.tile([65, 1024], F32, tag="cs", bufs=1, name="cs")
                pend = None  # (at, kc, live) awaiting its PV
                for kc in range(kcmax):
                    o = kc - 4 * qg
                    diag = causal and o >= 0
                    live = o * 128 if diag else 0
                    at = emit_scores(qkv, qg, kc, live, diag)
                    if pend is not None:
                        emit_pv(cs, vnb, *pend, first=(pend[1] == 0),
                                last=False)
                    pend = (at, kc, live)
                emit_pv(cs, vnb, *pend, first=(pend[1] == 0), last=True)
                # evict both heads in one op, then per-head recip + broadcast
                nc.vector.tensor_copy(
                    ex.rearrange("p (two t) -> p two t", two=2)
                    [:, :, qg * 512:(qg + 1) * 512],
                    cs.rearrange("p (two n) -> p two n", two=2))
                for h in range(2):
                    s0 = h * T + qg * 512
                    rcp = stream.tile([1, 512], F32, tag="rcp", bufs=2,
                                      name="rcp")
                    nc.vector.reciprocal(rcp[:], ex[64:65, s0:s0 + 512])
                    nc.gpsimd.partition_broadcast(bcast[:, s0:s0 + 512],
                                                  rcp[:])

            pend_store = {0: None, 1: None}

            def flush_store(h):
                if pend_store[h] is not None:
                    dst, ost = pend_store[h]
                    nc.gpsimd.dma_start(dst, ost[:])
                    pend_store[h] = None

            def emit_phase5(b, h, ex, bcast, dt2):
                flush_store(h)
                cvu = ex[0:64, h * T:(h + 1) * T].rearrange(
                    "p (r s2 two) -> p two s2 r", two=2, s2=8)
                bcu = bcast[:, h * T:(h + 1) * T].rearrange(
                    "p (r s2 two) -> p two s2 r", two=2, s2=8)
                yts = stream.tile([128, NDC * 128], F32R, tag="yt", bufs=2,
                                  name="yts")
                ytv = yts.rearrange("p (c r) -> p c r", r=128)
                nc.vector.tensor_mul(ytv[0:64, :, :], cvu[:, 0, :, :],
                                     bcu[:, 0, :, :])
                nc.vector.tensor_mul(ytv[64:128, :, :], cvu[:, 1, :, :],
                                     bcu[:, 1, :, :])
                ost = stream.tile([128, D], F32, tag="ost", bufs=2, name="ost")
                for og in range(2):
                    po = pp.tile([128, 512], F32, tag="mm", bufs=2, name="po")
                    for cc in range(NDC):
                        nc.tensor.matmul(
                            po[:], yts[:, cc * 128:(cc + 1) * 128],
                            wout_sb[:, cc * D + og * 512:cc * D + og * 512 + 512],
                            start=(cc == 0), stop=False)
                    nc.tensor.matmul(po[:], ones_row[:],
                                     bout_sb[0:1, og * 512:(og + 1) * 512],
                                     start=False, stop=True)
                    nc.vector.tensor_mul(
                        ost[:, og * 512:(og + 1) * 512], po[:],
                        dt2[:, og * 512:(og + 1) * 512])
                pend_store[h] = (out_d[b, h * 128:(h + 1) * 128, :], ost)

            # ================= pipelined schedule =================
            cur = alloc_qkv()
            for tg in range(NTG):
                emit_qkv_tg(cur, 0, tg)
            cur_vnb = emit_transposes(cur[2])

            for b in range(B):
                ex = perb.tile([65, 2 * T], F32, tag="ex", bufs=1, name="ex")
                bcast = perb.tile([64, 2 * T], F32, tag="bc", bufs=1,
                                  name="bcast")
                if b + 1 < B:
                    nxt = alloc_qkv()
                for qg in range(NQG):
                    emit_attn_qg(b, qg, cur, cur_vnb, ex, bcast)
                    if b + 1 < B:
                        emit_qkv_tg(nxt, b + 1, qg)
                dt2s = []
                for h in range(HP):
                    dt2 = stream.tile([128, D], BF, tag="dt", bufs=2,
                                      name="dt2")
                    nc.sync.dma_start(dt2[:],
                                      drop_d[b, h * 128:(h + 1) * 128, :])
                    dt2s.append(dt2)
                if b + 1 < B:
                    nxt_vnb = emit_transposes(nxt[2])
                for h in range(HP):
                    emit_phase5(b, h, ex, bcast, dt2s[h])
                if b + 1 < B:
                    cur, cur_vnb = nxt, nxt_vnb
            flush_store(0)
            flush_store(1)

    nc.compile()
    return nc


def _get_program(causal: bool):
    key = ("causal" if causal else "full")
    if key not in _cache:
        _cache[key] = _build(causal)
    return _cache[key]


def _host_fallback(x, attn_mask, Wq, bq, Wk, bk, Wv, bv, Wout, bout,
                   dropout_mask):
    x64 = x.astype(np.float32)
    Q = np.einsum("btd,hdk->bhtk", x64, Wq) + bq[None, :, None, :]
    K = np.einsum("btd,hdk->bhtk", x64, Wk) + bk[None, :, None, :]
    V = np.einsum("btd,hdv->bhtv", x64, Wv) + bv[None, :, None, :]
    scores = np.einsum("bhqk,bhmk->bhqm", Q, K) * SCALE + attn_mask
    scores = scores - scores.max(-1, keepdims=True)
    e = np.exp(scores)
    attn = e / e.sum(-1, keepdims=True)
    ctx = np.einsum("bhqm,bhmv->bhqv", attn, V).reshape(B, T, H * DV)
    out = ctx @ Wout.T + bout
    return (out * dropout_mask).astype(np.float32)


def _chunked128(w):
    """[D, M] -> [128, (dc M)] with row d = dc*128 + p."""
    d, m = w.shape
    return np.ascontiguousarray(
        w.reshape(d // 128, 128, m).transpose(1, 0, 2).reshape(128, -1))


def kernel(x, attn_mask, Wq, bq, Wk, bk, Wv, bv, Wout, bout, dropout_mask):
    from concourse.bass_utils import run_bass_kernel_spmd

    x = np.ascontiguousarray(x, np.float32)
    m = np.asarray(attn_mask, np.float32).reshape(T, T)

    # causality check on the actual mask tensor
    causal = bool((np.tril(m) == 0).all() and
                  (m[np.triu_indices(T, 1)] <= -1e8).all())

    # safety: cheap bound on max |scaled score| -> exp overflow guard
    xf = x.reshape(B * T, D)
    Qa = xf @ Wq.transpose(1, 0, 2).reshape(D, H * DK)
    Ka = xf @ Wk.transpose(1, 0, 2).reshape(D, H * DK)
    Qa = Qa.reshape(B * T, H, DK) + bq[None]
    Ka = Ka.reshape(B * T, H, DK) + bk[None]
    qn = np.linalg.norm(Qa, axis=2).max(0)     # per-head max row norm
    kn = np.linalg.norm(Ka, axis=2).max(0)
    bound = float(SCALE) * float((qn * kn).max())
    if bound > 50.0:
        return _host_fallback(x, attn_mask, Wq, bq, Wk, bk, Wv, bv, Wout,
                              bout, dropout_mask)

    nc = _get_program(causal)

    # xT [128, dc, B*T] bf16
    xT = np.ascontiguousarray(
        x.transpose(2, 0, 1).reshape(NDC, 128, B * T).transpose(1, 0, 2)
    ).astype(BF16)
    woutT = np.asarray(Wout, np.float32).T            # [f, o]
    wout_sb = _chunked128(woutT).astype(BF16)
    boutr = np.asarray(bout, np.float32).reshape(1, D)
    idm = np.eye(128, dtype=np.float32).astype(BF16)
    dmask1 = np.where(np.arange(128)[None, :] < np.arange(128)[:, None],
                      MASK_NEG, np.float32(0.0)).astype(np.float32)
    dmask = np.concatenate([dmask1, dmask1], axis=1)
    maskT = None if causal else np.ascontiguousarray(m.T * np.float32(8.0))
    drop = np.asarray(dropout_mask, np.float32).astype(BF16)

    in_maps = []
    for c in range(NCORES):
        h0, h1 = HP * c, HP * c + 1
        im = {
            "xT": xT,
            "wq": _chunked128(
                np.concatenate([Wq[h0], Wq[h1]], axis=1)).astype(np.float32),
            "wk": _chunked128(
                np.concatenate([Wk[h0], Wk[h1]], axis=1)).astype(np.float32),
            "wv": _chunked128(
                np.concatenate([Wv[h0], Wv[h1]], axis=1)).astype(np.float32),
            "bq": np.concatenate([bq[h0], bq[h1]]).reshape(128, 1)
                    .astype(np.float32),
            "bk": np.concatenate([bk[h0], bk[h1]]).reshape(128, 1)
                    .astype(np.float32),
            "bv": np.concatenate([bv[h0], bv[h1]]).reshape(128, 1)
                    .astype(np.float32),
            "wout": wout_sb,
            "bout": boutr,
            "onesr": np.ones((1, 128), np.float32),
            "drop": np.ascontiguousarray(drop[:, c * ROWS:(c + 1) * ROWS, :]),
            "idm": idm,
        }
        if causal:
            im["dmask"] = dmask
        else:
            im["maskT"] = maskT
        in_maps.append(im)

    res = run_bass_kernel_spmd(nc, in_maps, list(range(NCORES)))
    out = np.empty((B, T, D), np.float32)
    for c in range(NCORES):
        out[:, c * ROWS:(c + 1) * ROWS, :] = res.results[c]["out"]
    return out


# revision 34
# speedup vs baseline: 1.3386x; 1.1297x over previous
"""Trainium2 Bass kernel for nn_MultiHeadAttention_48395691492101.

Strategy: pure head-parallel sharding across 8 NeuronCores (2 heads/core).
Because the reference reshapes ctx [B,H,T,DV] -> [B,T,H*DV] WITHOUT
transposing, row-block t' in [h*128,(h+1)*128) of the reshaped tensor comes
entirely from head h.  Core c (heads 2c,2c+1) therefore owns output rows
[c*256,(c+1)*256) of every batch, and the output projection needs no
cross-core reduction at all.

Datapath (all matmuls bf16 inputs / fp32 PSUM accumulation):
- two heads fused per tile: scores/exp tiles are [128, 1024], the PV
  accumulator is [65, 1024] (64 V rows + a ones row that yields softmax
  denominators for free).
- causal: dead 128x512 blocks skipped, diagonal blocks get a triangular
  -8e9 add and column-restricted exp/PV (dead columns never touched).
- per-q-group normalization: PSUM->SBUF evict, DVE reciprocal of the sums
  row, gpsimd partition_broadcast, and the multiply folded into the Y^T
  gather copies that feed the output projection.
- flat software-pipelined emission: every chunk's PV (and, on the last
  chunk of a q-group, the accumulator evict/norm close plus boundary
  fillers - next batch's QKV t-group, V transposes one group late, the
  previous batch's output projection) is emitted only after the next
  chunk's scores, so the PE instruction stream never drains on exp or
  evict latency. Output stores are emitted one batch late so their
  semaphore waits never block a queue; a self-contained warmup matmul
  chain keeps the PE p-state ramp alive while the first x tiles stream.
"""

import sys

if "/opt/trn_rl_repo" not in sys.path:
    sys.path.insert(0, "/opt/trn_rl_repo")

import numpy as np
import ml_dtypes

BF16 = ml_dtypes.bfloat16
FP8E4 = ml_dtypes.float8_e4m3

B, T, D = 4, 2048, 1024
H, DK, DV = 16, 64, 64
SCALE = np.float32(1.0 / 8.0)
NCORES = 8
HP = H // NCORES          # heads per core = 2
ROWS = HP * (T * DV) // D  # output rows per head pair per batch = 256
NDC = D // 128            # 8 d-chunks
NTG = 4                   # t-groups of 512 for QKV
NQG = 4                   # q-groups of 512
NKC = T // 128            # 16 k-chunks
MASK_NEG = np.float32(-8.0e9)   # becomes -1e9 after *SCALE inside exp

# moving-operand dtype knobs: f32r avoids Ldweights instructions on the PE
# sequencer; bf16 halves DMA and is 1 cycle/row at any output width
XT_F32R = False
QT_F32R = False
AT_F32R = False
WOUT_F32R = False
XT_BUFS = 3
USE_FP8 = False   # fp8e4m3 + DoubleRow for QKV projections and score matmuls

_cache = {}


def _build(causal: bool, debug: bool = False):
    import concourse.tile as tile
    import concourse.mybir as mybir
    from concourse import bacc

    F32 = mybir.dt.float32
    F32R = mybir.dt.float32r
    BF = mybir.dt.bfloat16
    FP8 = mybir.dt.float8e4
    XT_DT = FP8 if USE_FP8 else (F32R if XT_F32R else BF)   # pairs with wq/wk/wv
    QT_DT = FP8 if USE_FP8 else (F32R if QT_F32R else BF)   # pairs with kt
    AT_DT = F32R if AT_F32R else BF      # pairs with vnb (and vt/id/tp chain)
    WOUT_DT = F32R if WOUT_F32R else BF  # pairs with yts
    W_DT = XT_DT
    KT_DT = QT_DT
    VNB_DT = AT_DT
    VT_DT = AT_DT
    ID_DT = AT_DT
    YT_DT = WOUT_DT
    Exp = mybir.ActivationFunctionType.Exp

    nc = bacc.Bacc("TRN2", target_bir_lowering=False, debug=False,
                   num_devices=NCORES)

    # host-prearranged layouts:
    #   fp8: xT [128, c2, i, B*T] with d = c2*256 + 2p + i, w* [128, (c2 i j)]
    #   else: xT [128, dc, B*T] with d = dc*128 + p, w* [128, (dc j)]
    if USE_FP8:
        xT_d = nc.dram_tensor("xT", [128, 4, 2, B * T], XT_DT,
                              kind="ExternalInput").ap()
    else:
        xT_d = nc.dram_tensor("xT", [128, NDC, B * T], XT_DT,
                              kind="ExternalInput").ap()
    wq_d = nc.dram_tensor("wq", [128, D], W_DT, kind="ExternalInput").ap()
    wk_d = nc.dram_tensor("wk", [128, D], W_DT, kind="ExternalInput").ap()
    wv_d = nc.dram_tensor("wv", [128, D], W_DT, kind="ExternalInput").ap()
    bq_d = nc.dram_tensor("bq", [128, 1], F32, kind="ExternalInput").ap()
    bk_d = nc.dram_tensor("bk", [128, 1], F32, kind="ExternalInput").ap()
    bv_d = nc.dram_tensor("bv", [128, 1], F32, kind="ExternalInput").ap()
    wout_d = nc.dram_tensor("wout", [128, NDC * D], WOUT_DT, kind="ExternalInput").ap()
    bout_d = nc.dram_tensor("bout", [1, D], F32R, kind="ExternalInput").ap()
    onesr_d = nc.dram_tensor("onesr", [1, 128], F32R, kind="ExternalInput").ap()
    drop_d = nc.dram_tensor("drop", [B, ROWS, D], BF, kind="ExternalInput").ap()
    id_d = nc.dram_tensor("idm", [128, 128], ID_DT, kind="ExternalInput").ap()
    if causal:
        dmask_d = nc.dram_tensor("dmask", [128, 256], F32,
                                 kind="ExternalInput").ap()
    else:
        maskT_d = nc.dram_tensor("maskT", [T, T], F32, kind="ExternalInput").ap()
    out_d = nc.dram_tensor("out", [B, ROWS, D], F32, kind="ExternalOutput").ap()

    with tile.TileContext(nc) as tc:
        with tc.tile_pool(name="const", bufs=1) as cpool, \
             tc.tile_pool(name="perb", bufs=1) as perb, \
             tc.tile_pool(name="stream", bufs=3) as stream, \
             tc.tile_pool(name="psum", bufs=1, space="PSUM") as pp:

            # ---- constants ----
            # gpsimd queue: QKV weights + small tensors (needed first)
            wq_sb = cpool.tile([128, D], W_DT, tag="wq")
            wk_sb = cpool.tile([128, D], W_DT, tag="wk")
            wv_sb = cpool.tile([128, D], W_DT, tag="wv")
            nc.gpsimd.dma_start(wq_sb[:], wq_d[:])
            nc.gpsimd.dma_start(wk_sb[:], wk_d[:])
            nc.gpsimd.dma_start(wv_sb[:], wv_d[:])
            bq_sb = cpool.tile([128, 1], F32, tag="bq")
            bk_sb = cpool.tile([128, 1], F32, tag="bk")
            bv_sb = cpool.tile([128, 1], F32, tag="bv")
            nc.gpsimd.dma_start(bq_sb[:], bq_d[:])
            nc.gpsimd.dma_start(bk_sb[:], bk_d[:])
            nc.gpsimd.dma_start(bv_sb[:], bv_d[:])
            id_sb = cpool.tile([128, 128], ID_DT, tag="idm")
            nc.gpsimd.dma_start(id_sb[:], id_d[:])
            if causal:
                dmask_sb = cpool.tile([128, 256], F32, tag="dmask")
                nc.gpsimd.dma_start(dmask_sb[:], dmask_d[:])
            # scalar queue: output-projection constants - loaded after the
            # prologue QKV so their transfers don't delay the first x tiles
            wout_sb = cpool.tile([128, NDC * D], WOUT_DT, tag="wout")
            bout_sb = cpool.tile([1, D], F32R, tag="bout")
            ones_row = cpool.tile([1, 128], F32R, tag="onesr")

            def emit_late_consts():
                # chunked so no single transfer hogs the DMA engines
                for cc in range(NDC):
                    nc.scalar.dma_start(wout_sb[:, cc * D:(cc + 1) * D],
                                        wout_d[:, cc * D:(cc + 1) * D])
                nc.scalar.dma_start(bout_sb[:], bout_d[:])
                nc.scalar.dma_start(ones_row[:], onesr_d[:])

            def alloc_qkv():
                qt = perb.tile([128, T], QT_DT, tag="qt", bufs=2, name="qt")
                kt = perb.tile([128, T], KT_DT, tag="kt", bufs=2, name="kt")
                vt = perb.tile([128, T], VT_DT, tag="vt", bufs=2, name="vt")
                if USE_FP8:
                    # head h on partitions [32h,32h+32); dk = 32*i + pp
                    qt8 = perb.tile([64, 2, T], QT_DT, tag="qt8", bufs=2,
                                    name="qt8")
                    kt8 = perb.tile([64, 2, T], QT_DT, tag="kt8", bufs=2,
                                    name="kt8")
                    return qt, kt, vt, qt8, kt8
                return qt, kt, vt

            def emit_remap(qkv):
                # partition fold [128,T] -> [64,2,T] via 4 SBUF->SBUF DMAs each
                qt, kt = qkv[0], qkv[1]
                qt8, kt8 = qkv[3], qkv[4]
                for pre, packed in ((qt, qt8), (kt, kt8)):
                    for h in range(2):
                        for i in range(2):
                            nc.sync.dma_start(
                                packed[32 * h:32 * h + 32, i, :],
                                pre[64 * h + 32 * i:64 * h + 32 * i + 32, :])

            def emit_qkv_tg(qkv, b, tg):
                qt, kt, vt = qkv[0], qkv[1], qkv[2]
                c0 = b * T + tg * 512
                if USE_FP8:
                    xt = stream.tile([128, 4, 2, 512], XT_DT, tag="xt",
                                     bufs=XT_BUFS, name="xt")
                    nc.sync.dma_start(xt[:], xT_d[:, :, :, c0:c0 + 512])
                else:
                    xt = stream.tile([128, NDC * 512], XT_DT, tag="xt",
                                     bufs=XT_BUFS, name="xt")
                    nc.sync.dma_start(
                        xt.rearrange("p (dc j) -> p dc j", j=512),
                        xT_d[:, :, c0:c0 + 512])
                for w_sb, bias_sb, dst in ((wq_sb, bq_sb, qt),
                                           (wk_sb, bk_sb, kt),
                                           (wv_sb, bv_sb, vt)):
                    ps = pp.tile([128, 512], F32, tag="mm", bufs=2, name="ps")
                    if USE_FP8:
                        wv8 = w_sb.rearrange("p (c2 i j) -> p c2 i j",
                                             c2=4, i=2)
                        for c2 in range(4):
                            nc.tensor.matmul(
                                ps[:], wv8[:, c2], xt[:, c2],
                                start=(c2 == 0), stop=(c2 == 3),
                                perf_mode=mybir.MatmulPerfMode.DoubleRow)
                    else:
                        for dc in range(NDC):
                            nc.tensor.matmul(
                                ps[:], w_sb[:, dc * 128:(dc + 1) * 128],
                                xt[:, dc * 512:(dc + 1) * 512],
                                start=(dc == 0), stop=(dc == NDC - 1))
                    nc.vector.tensor_scalar_add(
                        dst[:, tg * 512:(tg + 1) * 512], ps[:], bias_sb[:])

            def alloc_vnb():
                # vnb layout per k-chunk: [two heads][64 V rows + ones + pad]
                vnb = perb.tile([128, NKC * 132], VNB_DT, tag="vnb", bufs=2,
                                name="vnb")
                nc.vector.memset(
                    vnb.rearrange("p (c two w) -> p c two w", two=2, w=66)
                    [:, :, :, 64:65], 1.0)
                return vnb

            def emit_vchunks(vnb, vt, tg):
                for kc in range(4 * tg, 4 * tg + 4):
                    tp = pp.tile([128, 128], VT_DT, tag="mm", bufs=2, name="tp")
                    nc.tensor.transpose(tp[:], vt[:, kc * 128:(kc + 1) * 128],
                                        id_sb[:])
                    nc.vector.tensor_copy(
                        vnb.rearrange("p (c two w) -> p c two w", two=2, w=66)
                        [:, kc, :, 0:64],
                        tp[:].rearrange("p (two v) -> p two v", two=2))

            def emit_scores(qkv, qg, kc, live, diag):
                qt, kt = qkv[0], qkv[1]
                st = pp.tile([128, 1024], F32, tag="st", bufs=2, name="st")
                if USE_FP8:
                    qt8, kt8 = qkv[3], qkv[4]
                    for h in range(2):
                        nc.tensor.matmul(
                            st[:, h * 512 + live:(h + 1) * 512],
                            kt8[32 * h:32 * h + 32, :,
                                kc * 128:(kc + 1) * 128],
                            qt8[32 * h:32 * h + 32, :,
                                qg * 512 + live:(qg + 1) * 512],
                            start=True, stop=True,
                            perf_mode=mybir.MatmulPerfMode.DoubleRow)
                else:
                    for h in range(2):
                        nc.tensor.matmul(
                            st[:, h * 512 + live:(h + 1) * 512],
                            kt[64 * h:64 * h + 64, kc * 128:(kc + 1) * 128],
                            qt[64 * h:64 * h + 64,
                               qg * 512 + live:(qg + 1) * 512],
                            start=True, stop=True)
                if diag:
                    sdv = st.rearrange("p (two n) -> p two n", two=2)
                    nc.vector.tensor_add(
                        sdv[:, :, live:live + 128],
                        sdv[:, :, live:live + 128],
                        dmask_sb.rearrange("p (two n) -> p two n", two=2))
                elif not causal:
                    mt = stream.tile([128, 512], F32, tag="mt", bufs=3,
                                     name="mt")
                    nc.sync.dma_start(
                        mt[:], maskT_d[kc * 128:(kc + 1) * 128,
                                       qg * 512:(qg + 1) * 512])
                    nc.vector.tensor_add(st[:, 0:512], st[:, 0:512], mt[:])
                    nc.vector.tensor_add(st[:, 512:1024], st[:, 512:1024],
                                         mt[:])
                at = stream.tile([128, 1024], AT_DT, tag="at", bufs=4, name="at")
                nc.scalar.activation(
                    at.rearrange("p (two n) -> p two n", two=2)[:, :, live:512],
                    st.rearrange("p (two n) -> p two n", two=2)[:, :, live:512],
                    Exp, scale=float(SCALE))
                return at

            def emit_pv(cs, vnb, at, kc, live, first, last):
                for h in range(2):
                    nc.tensor.matmul(
                        cs[:, h * 512 + live:(h + 1) * 512],
                        vnb[:, kc * 132 + 66 * h:kc * 132 + 66 * h + 65],
                        at[:, h * 512 + live:(h + 1) * 512],
                        start=first, stop=last, skip_group_check=True)

            def emit_gather(ytss, ex, bcast, h, qg):
                # normalized Y^T gather for one (head, q-group) column slice
                r0, r1 = qg * 32, (qg + 1) * 32
                cvu = ex[0:64, h * T:(h + 1) * T].rearrange(
                    "p (r s2 two) -> p two s2 r", two=2, s2=8)[:, :, :, r0:r1]
                bcu = bcast[:, h * T:(h + 1) * T].rearrange(
                    "p (r s2 two) -> p two s2 r", two=2, s2=8)[:, :, :, r0:r1]
                ytv = ytss[h].rearrange("p (c r) -> p c r", r=128)
                nc.vector.tensor_mul(ytv[0:64, :, r0:r1], cvu[:, 0],
                                     bcu[:, 0])
                nc.vector.tensor_mul(ytv[64:128, :, r0:r1], cvu[:, 1],
                                     bcu[:, 1])

            def make_close(qg, cs, ex, bcast, ytss, tail=False):  # noqa: tail unused
                # evict + per-head reciprocal + broadcast + normalized gather
                def close():
                    if qg == NQG - 1:
                        # per-head evict: h0's norm chain starts earliest
                        for h in range(2):
                            s0 = h * T + qg * 512
                            nc.vector.tensor_copy(
                                ex[:, s0:s0 + 512],
                                cs[:, h * 512:(h + 1) * 512])
                            rcp = stream.tile([1, 512], F32, tag="rcp",
                                              bufs=2, name="rcp")
                            nc.vector.reciprocal(rcp[:],
                                                 ex[64:65, s0:s0 + 512])
                            nc.gpsimd.partition_broadcast(
                                bcast[:, s0:s0 + 512], rcp[:])
                    else:
                        nc.vector.tensor_copy(
                            ex.rearrange("p (two t) -> p two t", two=2)
                            [:, :, qg * 512:(qg + 1) * 512],
                            cs.rearrange("p (two n) -> p two n", two=2))
                        for h in range(2):
                            s0 = h * T + qg * 512
                            rcp = stream.tile([1, 512], F32, tag="rcp",
                                              bufs=2, name="rcp")
                            nc.vector.reciprocal(rcp[:],
                                                 ex[64:65, s0:s0 + 512])
                            nc.gpsimd.partition_broadcast(
                                bcast[:, s0:s0 + 512], rcp[:])
                        for h in range(2):
                            emit_gather(ytss, ex, bcast, h, qg)
                return close

            pend_store = {0: None, 1: None}

            def flush_store(h):
                if pend_store[h] is not None:
                    dst, ost = pend_store[h]
                    nc.sync.dma_start(dst, ost[:])
                    pend_store[h] = None

            def emit_phase5(b, h, ex, bcast, dt2s, ytss, final=False):
                dt2 = dt2s[h]
                yts = ytss[h]
                flush_store(h)
                emit_gather(ytss, ex, bcast, h, NQG - 1)
                ost = stream.tile([128, D], F32, tag="ost", bufs=2, name="ost")
                for og in range(2):
                    po = pp.tile([128, 512], F32, tag="mm", bufs=2, name="po")
                    for cc in range(NDC):
                        nc.tensor.matmul(
                            po[:], yts[:, cc * 128:(cc + 1) * 128],
                            wout_sb[:, cc * D + og * 512:cc * D + og * 512 + 512],
                            start=(cc == 0), stop=False)
                    nc.tensor.matmul(po[:], ones_row[:],
                                     bout_sb[0:1, og * 512:(og + 1) * 512],
                                     start=False, stop=True)
                    nc.vector.tensor_mul(
                        ost[:, og * 512:(og + 1) * 512], po[:],
                        dt2[:, og * 512:(og + 1) * 512])
                    if final:
                        nc.sync.dma_start(
                            out_d[b, h * 128:(h + 1) * 128,
                                  og * 512:(og + 1) * 512],
                            ost[:, og * 512:(og + 1) * 512])
                if not final:
                    pend_store[h] = (out_d[b, h * 128:(h + 1) * 128, :], ost)

            # PE p-state warmup: self-contained matmul chain (no DMA deps)
            # keeps the clock ramp alive while the first x tiles stream in
            warm_sb = cpool.tile([128, 512], BF, tag="warm")
            nc.vector.memset(warm_sb[:], 0.0)

            def emit_warmup(n):
                wp = pp.tile([128, 512], F32, tag="mm", bufs=2, name="wp")
                for i in range(n):
                    nc.tensor.matmul(wp[:], warm_sb[:, 0:128], warm_sb[:],
                                     start=True, stop=True)

            # ============ flat software-pipelined schedule ============
            # Each chunk's PV (and, on the last chunk of a q-group, the
            # evict/norm close plus the boundary fillers: next batch's QKV,
            # V-transposes, previous batch's output projection) is emitted
            # only AFTER the next chunk's scores, so the PE stream never
            # drains waiting on exp or the accumulator evict.
            cur = alloc_qkv()
            cur_vnb = alloc_vnb()
            emit_warmup(14)
            if USE_FP8:
                # fp8 remap needs the full qt/kt before attention
                for tg in range(NTG):
                    emit_qkv_tg(cur, 0, tg)
                    if tg > 0:
                        emit_vchunks(cur_vnb, cur[2], tg - 1)
                emit_remap(cur)
                emit_vchunks(cur_vnb, cur[2], NTG - 1)
            else:
                # attention q-group qg only needs t-groups <= qg of Q/K and
                # V chunks <= qg, so b0's tg2/tg3 become boundary fillers
                # inside its own early attention instead of serial prologue
                emit_qkv_tg(cur, 0, 0)
                emit_qkv_tg(cur, 0, 1)
                emit_vchunks(cur_vnb, cur[2], 0)

            carry = {"v": None}  # pending PV (+close/fillers) of prev chunk
            pend_vchunks = []    # deferred tg3 V-transposes of the next batch

            def flush_carry():
                c = carry["v"]
                if c is None:
                    return
                emit_pv(*c["pv"])
                if c["close"] is not None:
                    c["close"]()
                for f in c["fillers"]:
                    f()
                carry["v"] = None

            prev = None   # (b-1's ex, bcast, dt2s) - phase5 runs inside attn(b)
            for b in range(B):
                ex = perb.tile([65, 2 * T], F32, tag="ex", bufs=2, name="ex")
                bcast = perb.tile([64, 2 * T], F32, tag="bc", bufs=2,
                                  name="bcast")
                ytss = [stream.tile([128, NDC * 128], YT_DT, tag="yt",
                                    bufs=4, name="yts") for _ in range(HP)]
                if b + 1 < B:
                    nxt = alloc_qkv()
                    nxt_vnb = alloc_vnb()
                dt2s = []
                for qg in range(NQG):
                    kcmax = 4 * qg + 4 if causal else NKC
                    cs = pp.tile([65, 1024], F32, tag="cs", bufs=1, name="cs")
                    fillers = []
                    if qg == 0 and pend_vchunks:
                        fillers.extend(pend_vchunks)
                        pend_vchunks.clear()
                    if b == 0 and not USE_FP8:
                        if qg < 2:
                            fillers.append(
                                lambda cc=cur, tg=qg + 2:
                                    emit_qkv_tg(cc, 0, tg))
                            fillers.append(
                                lambda vb=cur_vnb, vt=cur[2], tg=qg + 1:
                                    emit_vchunks(vb, vt, tg))
                        elif qg == 2:
                            fillers.append(
                                lambda vb=cur_vnb, vt=cur[2]:
                                    emit_vchunks(vb, vt, 3))
                    if b == 0 and qg == 1:
                        fillers.append(emit_late_consts)
                    if b + 1 < B:
                        fillers.append(
                            lambda nx=nxt, bb=b + 1, tg=qg:
                                emit_qkv_tg(nx, bb, tg))
                        if qg > 0:
                            fillers.append(
                                lambda vb=nxt_vnb, vt=nxt[2], tg=qg - 1:
                                    emit_vchunks(vb, vt, tg))
                    p5qg = qg - 1
                    if prev is not None and 0 <= p5qg < HP:
                        fillers.append(
                            lambda bb=b - 1, h=p5qg, pv=prev:
                                emit_phase5(bb, h, *pv))
                    if qg == NQG - 1:
                        fillers.append(lambda: emit_warmup(3))
                        def _dt(b=b, dt2s=dt2s):
                            for h in range(HP):
                                dt2 = stream.tile([128, D], BF, tag="dt",
                                                  bufs=2, name="dt2")
                                nc.sync.dma_start(
                                    dt2[:],
                                    drop_d[b, h * 128:(h + 1) * 128, :])
                                dt2s.append(dt2)
                        fillers.append(_dt)
                        if USE_FP8 and b + 1 < B:
                            fillers.append(lambda nx=nxt: emit_remap(nx))
                        if b + 1 < B:
                            pend_vchunks.append(
                                lambda vb=nxt_vnb, vt=nxt[2]:
                                    emit_vchunks(vb, vt, NTG - 1))
                    close = make_close(qg, cs, ex, bcast, ytss,
                                       tail=(b == B - 1))
                    for kc in range(kcmax):
                        o = kc - 4 * qg
                        diag = causal and o >= 0
                        live = o * 128 if diag else 0
                        at = emit_scores(cur, qg, kc, live, diag)
                        flush_carry()
                        if b == B - 1 and qg == NQG - 1 and kc % 4 == 2:
                            emit_warmup(1)
                        last = (kc == kcmax - 1)
                        carry["v"] = {
                            "pv": (cs, cur_vnb, at, kc, live, kc == 0, last),
                            "close": close if last else None,
                            "fillers": fillers if last else [],
                        }
                prev = (ex, bcast, dt2s, ytss)
                if b + 1 < B:
                    cur, cur_vnb = nxt, nxt_vnb
            flush_carry()
            emit_warmup(12)
            for h in range(HP):
                emit_phase5(B - 1, h, *prev, final=True)
            flush_store(0)
            flush_store(1)

    nc.compile()
    return nc


def _get_program(causal: bool):
    key = ("causal" if causal else "full")
    if key not in _cache:
        _cache[key] = _build(causal)
    return _cache[key]


def _host_fallback(x, attn_mask, Wq, bq, Wk, bk, Wv, bv, Wout, bout,
                   dropout_mask):
    x64 = x.astype(np.float32)
    Q = np.einsum("btd,hdk->bhtk", x64, Wq) + bq[None, :, None, :]
    K = np.einsum("btd,hdk->bhtk", x64, Wk) + bk[None, :, None, :]
    V = np.einsum("btd,hdv->bhtv", x64, Wv) + bv[None, :, None, :]
    scores = np.einsum("bhqk,bhmk->bhqm", Q, K) * SCALE + attn_mask
    scores = scores - scores.max(-1, keepdims=True)
    e = np.exp(scores)
    attn = e / e.sum(-1, keepdims=True)
    ctx = np.einsum("bhqm,bhmv->bhqv", attn, V).reshape(B, T, H * DV)
    out = ctx @ Wout.T + bout
    return (out * dropout_mask).astype(np.float32)


def _chunked128(w):
    """[D, M] -> [128, (dc M)] with row d = dc*128 + p."""
    d, m = w.shape
    return np.ascontiguousarray(
        w.reshape(d // 128, 128, m).transpose(1, 0, 2).reshape(128, -1))


def kernel(x, attn_mask, Wq, bq, Wk, bk, Wv, bv, Wout, bout, dropout_mask):
    from concourse.bass_utils import run_bass_kernel_spmd

    x = np.ascontiguousarray(x, np.float32)
    m = np.asarray(attn_mask, np.float32).reshape(T, T)

    # causality check on the actual mask tensor
    causal = bool((np.tril(m) == 0).all() and
                  (m[np.triu_indices(T, 1)] <= -1e8).all())

    # safety: cheap bound on max |scaled score| -> exp overflow guard
    xf = x.reshape(B * T, D)
    Qa = xf @ Wq.transpose(1, 0, 2).reshape(D, H * DK)
    Ka = xf @ Wk.transpose(1, 0, 2).reshape(D, H * DK)
    Qa = Qa.reshape(B * T, H, DK) + bq[None]
    Ka = Ka.reshape(B * T, H, DK) + bk[None]
    qn = np.linalg.norm(Qa, axis=2).max(0)     # per-head max row norm
    kn = np.linalg.norm(Ka, axis=2).max(0)
    bound = float(SCALE) * float((qn * kn).max())
    if bound > 50.0:
        return _host_fallback(x, attn_mask, Wq, bq, Wk, bk, Wv, bv, Wout,
                              bout, dropout_mask)

    nc = _get_program(causal)

    xd = x.transpose(2, 0, 1).reshape(D, B * T)
    if USE_FP8:
        # [128, c2, i, B*T] with d = c2*256 + 2p + i
        xT = np.ascontiguousarray(
            xd.reshape(4, 128, 2, B * T).transpose(1, 0, 2, 3)).astype(FP8E4)
    else:
        xT = np.ascontiguousarray(
            xd.reshape(NDC, 128, B * T).transpose(1, 0, 2)
        ).astype(np.float32 if XT_F32R else BF16)
    woutT = np.asarray(Wout, np.float32).T            # [f, o]
    wout_sb = _chunked128(woutT).astype(np.float32 if WOUT_F32R else BF16)
    boutr = np.asarray(bout, np.float32).reshape(1, D)
    _w_np = np.float32 if XT_F32R else BF16

    def _wpack(w):   # [D, 128] -> [128, D] in the matching matmul layout
        if USE_FP8:
            return np.ascontiguousarray(
                w.reshape(4, 128, 2, 128).transpose(1, 0, 2, 3)
                .reshape(128, D)).astype(FP8E4)
        return _chunked128(w).astype(_w_np)
    idm = np.eye(128, dtype=np.float32) if AT_F32R else np.eye(128, dtype=np.float32).astype(BF16)
    dmask1 = np.where(np.arange(128)[None, :] < np.arange(128)[:, None],
                      MASK_NEG, np.float32(0.0)).astype(np.float32)
    dmask = np.concatenate([dmask1, dmask1], axis=1)
    maskT = None if causal else np.ascontiguousarray(m.T * np.float32(8.0))
    drop = np.asarray(dropout_mask, np.float32).astype(BF16)

    in_maps = []
    for c in range(NCORES):
        h0, h1 = HP * c, HP * c + 1
        im = {
            "xT": xT,
            "wq": _wpack(np.concatenate([Wq[h0], Wq[h1]], axis=1)),
            "wk": _wpack(np.concatenate([Wk[h0], Wk[h1]], axis=1)),
            "wv": _wpack(np.concatenate([Wv[h0], Wv[h1]], axis=1)),
            "bq": np.concatenate([bq[h0], bq[h1]]).reshape(128, 1)
                    .astype(np.float32),
            "bk": np.concatenate([bk[h0], bk[h1]]).reshape(128, 1)
                    .astype(np.float32),
            "bv": np.concatenate([bv[h0], bv[h1]]).reshape(128, 1)
                    .astype(np.float32),
            "wout": wout_sb,
            "bout": boutr,
            "onesr": np.ones((1, 128), np.float32),
            "drop": np.ascontiguousarray(drop[:, c * ROWS:(c + 1) * ROWS, :]),
            "idm": idm,
        }
        if causal:
            im["dmask"] = dmask
        else:
            im["maskT"] = maskT
        in_maps.append(im)

    res = run_bass_kernel_spmd(nc, in_maps, list(range(NCORES)))
    out = np.empty((B, T, D), np.float32)
    for c in range(NCORES):
        out[:, c * ROWS:(c + 1) * ROWS, :] = res.results[c]["out"]
    return out


# revision 38
# speedup vs baseline: 1.3998x; 1.0457x over previous
"""Trainium2 Bass kernel for nn_MultiHeadAttention_48395691492101.

Strategy: pure head-parallel sharding across 8 NeuronCores (2 heads/core).
Because the reference reshapes ctx [B,H,T,DV] -> [B,T,H*DV] WITHOUT
transposing, row-block t' in [h*128,(h+1)*128) of the reshaped tensor comes
entirely from head h.  Core c (heads 2c,2c+1) therefore owns output rows
[c*256,(c+1)*256) of every batch, and the output projection needs no
cross-core reduction at all.

Datapath (all matmuls bf16 inputs / fp32 PSUM accumulation):
- two heads fused per tile: scores/exp tiles are [128, 1024], the PV
  accumulator is [65, 1024] (64 V rows + a ones row that yields softmax
  denominators for free).
- causal: dead 128x512 blocks skipped, diagonal blocks get a triangular
  -8e9 add and column-restricted exp/PV (dead columns never touched).
- per-q-group normalization: PSUM->SBUF evict, DVE reciprocal of the sums
  row, gpsimd partition_broadcast, and the multiply folded into the Y^T
  gather copies that feed the output projection.
- flat software-pipelined emission: every chunk's PV (and, on the last
  chunk of a q-group, the accumulator evict/norm close plus boundary
  fillers - next batch's QKV t-group, V transposes one group late, the
  previous batch's output projection) is emitted only after the next
  chunk's scores, so the PE instruction stream never drains on exp or
  evict latency. Output stores are emitted one batch late so their
  semaphore waits never block a queue; a self-contained warmup matmul
  chain keeps the PE p-state ramp alive while the first x tiles stream.
"""

import sys

if "/opt/trn_rl_repo" not in sys.path:
    sys.path.insert(0, "/opt/trn_rl_repo")

import numpy as np
import ml_dtypes

BF16 = ml_dtypes.bfloat16
FP8E4 = ml_dtypes.float8_e4m3

B, T, D = 4, 2048, 1024
H, DK, DV = 16, 64, 64
SCALE = np.float32(1.0 / 8.0)
NCORES = 8
HP = H // NCORES          # heads per core = 2
ROWS = HP * (T * DV) // D  # output rows per head pair per batch = 256
NDC = D // 128            # 8 d-chunks
NTG = 4                   # t-groups of 512 for QKV
NQG = 4                   # q-groups of 512
NKC = T // 128            # 16 k-chunks
MASK_NEG = np.float32(-8.0e9)   # becomes -1e9 after *SCALE inside exp

# moving-operand dtype knobs: f32r avoids Ldweights instructions on the PE
# sequencer; bf16 halves DMA and is 1 cycle/row at any output width
XT_F32R = False
QT_F32R = False
AT_F32R = False
WOUT_F32R = False
XT_BUFS = 3
USE_FP8 = False   # fp8e4m3 + DoubleRow for QKV projections and score matmuls

_cache = {}


def _build(causal: bool, debug: bool = False):
    import concourse.tile as tile
    import concourse.mybir as mybir
    from concourse import bacc

    F32 = mybir.dt.float32
    F32R = mybir.dt.float32r
    BF = mybir.dt.bfloat16
    FP8 = mybir.dt.float8e4
    XT_DT = FP8 if USE_FP8 else (F32R if XT_F32R else BF)   # pairs with wq/wk/wv
    QT_DT = FP8 if USE_FP8 else (F32R if QT_F32R else BF)   # pairs with kt
    AT_DT = F32R if AT_F32R else BF      # pairs with vnb (and vt/id/tp chain)
    WOUT_DT = F32R if WOUT_F32R else BF  # pairs with yts
    W_DT = XT_DT
    KT_DT = QT_DT
    VNB_DT = AT_DT
    VT_DT = AT_DT
    ID_DT = AT_DT
    YT_DT = WOUT_DT
    Exp = mybir.ActivationFunctionType.Exp

    nc = bacc.Bacc("TRN2", target_bir_lowering=False, debug=False,
                   num_devices=NCORES)

    # host-prearranged layouts:
    #   fp8: xT [128, c2, i, B*T] with d = c2*256 + 2p + i, w* [128, (c2 i j)]
    #   else: xT [128, dc, B*T] with d = dc*128 + p, w* [128, (dc j)]
    if USE_FP8:
        xT_d = nc.dram_tensor("xT", [128, 4, 2, B * T], XT_DT,
                              kind="ExternalInput").ap()
    else:
        xT_d = nc.dram_tensor("xT", [128, NDC, B * T], XT_DT,
                              kind="ExternalInput").ap()
    wq_d = nc.dram_tensor("wq", [128, D], W_DT, kind="ExternalInput").ap()
    wk_d = nc.dram_tensor("wk", [128, D], W_DT, kind="ExternalInput").ap()
    wv_d = nc.dram_tensor("wv", [128, D], W_DT, kind="ExternalInput").ap()
    bq_d = nc.dram_tensor("bq", [128, 1], F32, kind="ExternalInput").ap()
    bk_d = nc.dram_tensor("bk", [128, 1], F32, kind="ExternalInput").ap()
    bv_d = nc.dram_tensor("bv", [128, 1], F32, kind="ExternalInput").ap()
    wout_d = nc.dram_tensor("wout", [128, NDC * D], WOUT_DT, kind="ExternalInput").ap()
    bout_d = nc.dram_tensor("bout", [1, D], F32R, kind="ExternalInput").ap()
    onesr_d = nc.dram_tensor("onesr", [1, 128], F32R, kind="ExternalInput").ap()
    drop_d = nc.dram_tensor("drop", [B, ROWS, D], BF, kind="ExternalInput").ap()
    id_d = nc.dram_tensor("idm", [128, 128], ID_DT, kind="ExternalInput").ap()
    if causal:
        dmask_d = nc.dram_tensor("dmask", [128, 256], F32,
                                 kind="ExternalInput").ap()
    else:
        maskT_d = nc.dram_tensor("maskT", [T, T], F32, kind="ExternalInput").ap()
    out_d = nc.dram_tensor("out", [B, ROWS, D], F32, kind="ExternalOutput").ap()

    with tile.TileContext(nc) as tc:
        with tc.tile_pool(name="const", bufs=1) as cpool, \
             tc.tile_pool(name="perb", bufs=1) as perb, \
             tc.tile_pool(name="stream", bufs=3) as stream, \
             tc.tile_pool(name="psum", bufs=1, space="PSUM") as pp:

            # ---- constants ----
            # gpsimd queue: QKV weights + small tensors (needed first)
            wq_sb = cpool.tile([128, D], W_DT, tag="wq")
            wk_sb = cpool.tile([128, D], W_DT, tag="wk")
            wv_sb = cpool.tile([128, D], W_DT, tag="wv")
            nc.gpsimd.dma_start(wq_sb[:], wq_d[:])
            nc.gpsimd.dma_start(wk_sb[:], wk_d[:])
            nc.gpsimd.dma_start(wv_sb[:], wv_d[:])
            bq_sb = cpool.tile([128, 1], F32, tag="bq")
            bk_sb = cpool.tile([128, 1], F32, tag="bk")
            bv_sb = cpool.tile([128, 1], F32, tag="bv")
            nc.gpsimd.dma_start(bq_sb[:], bq_d[:])
            nc.gpsimd.dma_start(bk_sb[:], bk_d[:])
            nc.gpsimd.dma_start(bv_sb[:], bv_d[:])
            id_sb = cpool.tile([128, 128], ID_DT, tag="idm")
            nc.gpsimd.dma_start(id_sb[:], id_d[:])
            if causal:
                dmask_sb = cpool.tile([128, 256], F32, tag="dmask")
                nc.gpsimd.dma_start(dmask_sb[:], dmask_d[:])
            # scalar queue: output-projection constants - loaded after the
            # prologue QKV so their transfers don't delay the first x tiles
            wout_sb = cpool.tile([128, NDC * D], WOUT_DT, tag="wout")
            bout_sb = cpool.tile([1, D], F32R, tag="bout")
            ones_row = cpool.tile([1, 128], F32R, tag="onesr")

            def emit_late_consts():
                # chunked so no single transfer hogs the DMA engines
                for cc in range(NDC):
                    nc.scalar.dma_start(wout_sb[:, cc * D:(cc + 1) * D],
                                        wout_d[:, cc * D:(cc + 1) * D])
                nc.scalar.dma_start(bout_sb[:], bout_d[:])
                nc.scalar.dma_start(ones_row[:], onesr_d[:])

            def alloc_qkv():
                qt = perb.tile([128, T], QT_DT, tag="qt", bufs=2, name="qt")
                kt = perb.tile([128, T], KT_DT, tag="kt", bufs=2, name="kt")
                vt = perb.tile([128, T], VT_DT, tag="vt", bufs=2, name="vt")
                if USE_FP8:
                    # head h on partitions [32h,32h+32); dk = 32*i + pp
                    qt8 = perb.tile([64, 2, T], QT_DT, tag="qt8", bufs=2,
                                    name="qt8")
                    kt8 = perb.tile([64, 2, T], QT_DT, tag="kt8", bufs=2,
                                    name="kt8")
                    return qt, kt, vt, qt8, kt8
                return qt, kt, vt

            def emit_remap(qkv):
                # partition fold [128,T] -> [64,2,T] via 4 SBUF->SBUF DMAs each
                qt, kt = qkv[0], qkv[1]
                qt8, kt8 = qkv[3], qkv[4]
                for pre, packed in ((qt, qt8), (kt, kt8)):
                    for h in range(2):
                        for i in range(2):
                            nc.sync.dma_start(
                                packed[32 * h:32 * h + 32, i, :],
                                pre[64 * h + 32 * i:64 * h + 32 * i + 32, :])

            def emit_qkv_tg(qkv, b, tg):
                qt, kt, vt = qkv[0], qkv[1], qkv[2]
                c0 = b * T + tg * 512
                if USE_FP8:
                    xt = stream.tile([128, 4, 2, 512], XT_DT, tag="xt",
                                     bufs=XT_BUFS, name="xt")
                    nc.sync.dma_start(xt[:], xT_d[:, :, :, c0:c0 + 512])
                else:
                    xt = stream.tile([128, NDC * 512], XT_DT, tag="xt",
                                     bufs=XT_BUFS, name="xt")
                    nc.sync.dma_start(
                        xt.rearrange("p (dc j) -> p dc j", j=512),
                        xT_d[:, :, c0:c0 + 512])
                for w_sb, bias_sb, dst in ((wq_sb, bq_sb, qt),
                                           (wk_sb, bk_sb, kt),
                                           (wv_sb, bv_sb, vt)):
                    ps = pp.tile([128, 512], F32, tag="mm", bufs=2, name="ps")
                    if USE_FP8:
                        wv8 = w_sb.rearrange("p (c2 i j) -> p c2 i j",
                                             c2=4, i=2)
                        for c2 in range(4):
                            nc.tensor.matmul(
                                ps[:], wv8[:, c2], xt[:, c2],
                                start=(c2 == 0), stop=(c2 == 3),
                                perf_mode=mybir.MatmulPerfMode.DoubleRow)
                    else:
                        for dc in range(NDC):
                            nc.tensor.matmul(
                                ps[:], w_sb[:, dc * 128:(dc + 1) * 128],
                                xt[:, dc * 512:(dc + 1) * 512],
                                start=(dc == 0), stop=(dc == NDC - 1))
                    nc.vector.tensor_scalar_add(
                        dst[:, tg * 512:(tg + 1) * 512], ps[:], bias_sb[:])

            def alloc_vnb():
                # vnb layout per k-chunk: [two heads][64 V rows + ones + pad]
                vnb = perb.tile([128, NKC * 132], VNB_DT, tag="vnb", bufs=2,
                                name="vnb")
                nc.vector.memset(
                    vnb.rearrange("p (c two w) -> p c two w", two=2, w=66)
                    [:, :, :, 64:65], 1.0)
                return vnb

            def emit_vchunks(vnb, vt, tg):
                for kc in range(4 * tg, 4 * tg + 4):
                    tp = pp.tile([128, 128], VT_DT, tag="mm", bufs=2, name="tp")
                    nc.tensor.transpose(tp[:], vt[:, kc * 128:(kc + 1) * 128],
                                        id_sb[:])
                    nc.vector.tensor_copy(
                        vnb.rearrange("p (c two w) -> p c two w", two=2, w=66)
                        [:, kc, :, 0:64],
                        tp[:].rearrange("p (two v) -> p two v", two=2))

            def emit_scores(qkv, qg, kc, live, diag):
                qt, kt = qkv[0], qkv[1]
                st = pp.tile([128, 1024], F32, tag="st", bufs=2, name="st")
                if USE_FP8:
                    qt8, kt8 = qkv[3], qkv[4]
                    for h in range(2):
                        nc.tensor.matmul(
                            st[:, h * 512 + live:(h + 1) * 512],
                            kt8[32 * h:32 * h + 32, :,
                                kc * 128:(kc + 1) * 128],
                            qt8[32 * h:32 * h + 32, :,
                                qg * 512 + live:(qg + 1) * 512],
                            start=True, stop=True,
                            perf_mode=mybir.MatmulPerfMode.DoubleRow)
                else:
                    for h in range(2):
                        nc.tensor.matmul(
                            st[:, h * 512 + live:(h + 1) * 512],
                            kt[64 * h:64 * h + 64, kc * 128:(kc + 1) * 128],
                            qt[64 * h:64 * h + 64,
                               qg * 512 + live:(qg + 1) * 512],
                            start=True, stop=True)
                if diag:
                    sdv = st.rearrange("p (two n) -> p two n", two=2)
                    nc.vector.tensor_add(
                        sdv[:, :, live:live + 128],
                        sdv[:, :, live:live + 128],
                        dmask_sb.rearrange("p (two n) -> p two n", two=2))
                elif not causal:
                    mt = stream.tile([128, 512], F32, tag="mt", bufs=3,
                                     name="mt")
                    nc.sync.dma_start(
                        mt[:], maskT_d[kc * 128:(kc + 1) * 128,
                                       qg * 512:(qg + 1) * 512])
                    nc.vector.tensor_add(st[:, 0:512], st[:, 0:512], mt[:])
                    nc.vector.tensor_add(st[:, 512:1024], st[:, 512:1024],
                                         mt[:])
                at = stream.tile([128, 1024], AT_DT, tag="at", bufs=4, name="at")
                nc.scalar.activation(
                    at.rearrange("p (two n) -> p two n", two=2)[:, :, live:512],
                    st.rearrange("p (two n) -> p two n", two=2)[:, :, live:512],
                    Exp, scale=float(SCALE))
                return at

            def emit_pv(cs, vnb, at, kc, live, first, last):
                for h in range(2):
                    nc.tensor.matmul(
                        cs[:, h * 512 + live:(h + 1) * 512],
                        vnb[:, kc * 132 + 66 * h:kc * 132 + 66 * h + 65],
                        at[:, h * 512 + live:(h + 1) * 512],
                        start=first, stop=last, skip_group_check=True)

            def emit_gather(ytss, ex, bcast, h, qg):
                # normalized Y^T gather for one (head, q-group) column slice
                r0, r1 = qg * 32, (qg + 1) * 32
                cvu = ex[0:64, h * T:(h + 1) * T].rearrange(
                    "p (r s2 two) -> p two s2 r", two=2, s2=8)[:, :, :, r0:r1]
                bcu = bcast[:, h * T:(h + 1) * T].rearrange(
                    "p (r s2 two) -> p two s2 r", two=2, s2=8)[:, :, :, r0:r1]
                ytv = ytss[h].rearrange("p (c r) -> p c r", r=128)
                nc.vector.tensor_mul(ytv[0:64, :, r0:r1], cvu[:, 0],
                                     bcu[:, 0])
                nc.vector.tensor_mul(ytv[64:128, :, r0:r1], cvu[:, 1],
                                     bcu[:, 1])

            def make_close(qg, cs, ex, bcast, ytss, tail=False):  # noqa: tail unused
                # evict + per-head reciprocal + broadcast + normalized gather
                def close():
                    if qg == NQG - 1:
                        # per-head evict on the scalar engine: its queue is
                        # idle at batch boundaries, so the evict dispatches
                        # immediately instead of queueing behind DVE work,
                        # and the DVE reciprocals/gathers run in parallel
                        for h in range(2):
                            s0 = h * T + qg * 512
                            nc.scalar.copy(ex[:, s0:s0 + 512],
                                           cs[:, h * 512:(h + 1) * 512])
                            rcp = stream.tile([1, 512], F32, tag="rcp",
                                              bufs=2, name="rcp")
                            nc.vector.reciprocal(rcp[:],
                                                 ex[64:65, s0:s0 + 512])
                            nc.gpsimd.partition_broadcast(
                                bcast[:, s0:s0 + 512], rcp[:])
                    else:
                        nc.vector.tensor_copy(
                            ex.rearrange("p (two t) -> p two t", two=2)
                            [:, :, qg * 512:(qg + 1) * 512],
                            cs.rearrange("p (two n) -> p two n", two=2))
                        for h in range(2):
                            s0 = h * T + qg * 512
                            rcp = stream.tile([1, 512], F32, tag="rcp",
                                              bufs=2, name="rcp")
                            nc.vector.reciprocal(rcp[:],
                                                 ex[64:65, s0:s0 + 512])
                            nc.gpsimd.partition_broadcast(
                                bcast[:, s0:s0 + 512], rcp[:])
                        for h in range(2):
                            emit_gather(ytss, ex, bcast, h, qg)
                return close

            pend_store = {0: None, 1: None}

            def flush_store(h):
                if pend_store[h] is not None:
                    dst, ost = pend_store[h]
                    nc.sync.dma_start(dst, ost[:])
                    pend_store[h] = None

            def emit_phase5(b, h, ex, bcast, dt2s, ytss, final=False):
                dt2 = dt2s[h]
                yts = ytss[h]
                flush_store(h)
                emit_gather(ytss, ex, bcast, h, NQG - 1)
                ost = stream.tile([128, D], F32, tag="ost", bufs=2, name="ost")
                for og in range(2):
                    po = pp.tile([128, 512], F32, tag="mm", bufs=2, name="po")
                    for cc in range(NDC):
                        nc.tensor.matmul(
                            po[:], yts[:, cc * 128:(cc + 1) * 128],
                            wout_sb[:, cc * D + og * 512:cc * D + og * 512 + 512],
                            start=(cc == 0), stop=False)
                    nc.tensor.matmul(po[:], ones_row[:],
                                     bout_sb[0:1, og * 512:(og + 1) * 512],
                                     start=False, stop=True)
                    nc.vector.tensor_mul(
                        ost[:, og * 512:(og + 1) * 512], po[:],
                        dt2[:, og * 512:(og + 1) * 512])
                    if final:
                        nc.sync.dma_start(
                            out_d[b, h * 128:(h + 1) * 128,
                                  og * 512:(og + 1) * 512],
                            ost[:, og * 512:(og + 1) * 512])
                if not final:
                    pend_store[h] = (out_d[b, h * 128:(h + 1) * 128, :], ost)

            # PE p-state warmup: self-contained matmul chain (no DMA deps)
            # keeps the clock ramp alive while the first x tiles stream in
            warm_sb = cpool.tile([128, 512], BF, tag="warm")
            nc.vector.memset(warm_sb[:], 0.0)

            def emit_warmup(n):
                wp = pp.tile([128, 512], F32, tag="mm", bufs=2, name="wp")
                for i in range(n):
                    nc.tensor.matmul(wp[:], warm_sb[:, 0:128], warm_sb[:],
                                     start=True, stop=True)

            # ============ flat software-pipelined schedule ============
            # Each chunk's PV (and, on the last chunk of a q-group, the
            # evict/norm close plus the boundary fillers: next batch's QKV,
            # V-transposes, previous batch's output projection) is emitted
            # only AFTER the next chunk's scores, so the PE stream never
            # drains waiting on exp or the accumulator evict.
            cur = alloc_qkv()
            cur_vnb = alloc_vnb()
            emit_warmup(14)
            if USE_FP8:
                # fp8 remap needs the full qt/kt before attention
                for tg in range(NTG):
                    emit_qkv_tg(cur, 0, tg)
                    if tg > 0:
                        emit_vchunks(cur_vnb, cur[2], tg - 1)
                emit_remap(cur)
                emit_vchunks(cur_vnb, cur[2], NTG - 1)
            else:
                # attention q-group qg only needs t-groups <= qg of Q/K and
                # V chunks <= qg, so b0's tg2/tg3 become boundary fillers
                # inside its own early attention instead of serial prologue
                emit_qkv_tg(cur, 0, 0)
                emit_qkv_tg(cur, 0, 1)
                emit_vchunks(cur_vnb, cur[2], 0)

            carry = {"v": None}  # pending PV (+close/fillers) of prev chunk
            pend_vchunks = []    # deferred tg3 V-transposes of the next batch

            def flush_carry():
                c = carry["v"]
                if c is None:
                    return
                emit_pv(*c["pv"])
                if c["close"] is not None:
                    c["close"]()
                for f in c["fillers"]:
                    f()
                carry["v"] = None

            prev = None   # (b-1's ex, bcast, dt2s) - phase5 runs inside attn(b)
            for b in range(B):
                ex = perb.tile([65, 2 * T], F32, tag="ex", bufs=2, name="ex")
                bcast = perb.tile([64, 2 * T], F32, tag="bc", bufs=2,
                                  name="bcast")
                ytss = [stream.tile([128, NDC * 128], YT_DT, tag="yt",
                                    bufs=4, name="yts") for _ in range(HP)]
                if b + 1 < B:
                    nxt = alloc_qkv()
                    nxt_vnb = alloc_vnb()
                dt2s = []
                for qg in range(NQG):
                    kcmax = 4 * qg + 4 if causal else NKC
                    cs = pp.tile([65, 1024], F32, tag="cs", bufs=1, name="cs")
                    fillers = []
                    if qg == 0 and pend_vchunks:
                        fillers.extend(pend_vchunks)
                        pend_vchunks.clear()
                    if b == 0 and not USE_FP8:
                        if qg < 2:
                            fillers.append(
                                lambda cc=cur, tg=qg + 2:
                                    emit_qkv_tg(cc, 0, tg))
                            fillers.append(
                                lambda vb=cur_vnb, vt=cur[2], tg=qg + 1:
                                    emit_vchunks(vb, vt, tg))
                        elif qg == 2:
                            fillers.append(
                                lambda vb=cur_vnb, vt=cur[2]:
                                    emit_vchunks(vb, vt, 3))
                    if b == 0 and qg == 1:
                        fillers.append(emit_late_consts)
                    if b + 1 < B:
                        fillers.append(
                            lambda nx=nxt, bb=b + 1, tg=qg:
                                emit_qkv_tg(nx, bb, tg))
                        if qg > 0:
                            fillers.append(
                                lambda vb=nxt_vnb, vt=nxt[2], tg=qg - 1:
                                    emit_vchunks(vb, vt, tg))
                    p5qg = qg - 1
                    if prev is not None and 0 <= p5qg < HP:
                        fillers.append(
                            lambda bb=b - 1, h=p5qg, pv=prev:
                                emit_phase5(bb, h, *pv))
                    if qg == NQG - 1:
                        def _dt(b=b, dt2s=dt2s):
                            for h in range(HP):
                                dt2 = stream.tile([128, D], BF, tag="dt",
                                                  bufs=2, name="dt2")
                                nc.sync.dma_start(
                                    dt2[:],
                                    drop_d[b, h * 128:(h + 1) * 128, :])
                                dt2s.append(dt2)
                        fillers.append(_dt)
                        if USE_FP8 and b + 1 < B:
                            fillers.append(lambda nx=nxt: emit_remap(nx))
                        if b + 1 < B:
                            pend_vchunks.append(
                                lambda vb=nxt_vnb, vt=nxt[2]:
                                    emit_vchunks(vb, vt, NTG - 1))
                    close = make_close(qg, cs, ex, bcast, ytss,
                                       tail=(b == B - 1))
                    for kc in range(kcmax):
                        o = kc - 4 * qg
                        diag = causal and o >= 0
                        live = o * 128 if diag else 0
                        at = emit_scores(cur, qg, kc, live, diag)
                        flush_carry()
                        last = (kc == kcmax - 1)
                        carry["v"] = {
                            "pv": (cs, cur_vnb, at, kc, live, kc == 0, last),
                            "close": close if last else None,
                            "fillers": fillers if last else [],
                        }
                prev = (ex, bcast, dt2s, ytss)
                if b + 1 < B:
                    cur, cur_vnb = nxt, nxt_vnb
            flush_carry()
            for h in range(HP):
                emit_phase5(B - 1, h, *prev, final=True)
            flush_store(0)
            flush_store(1)

    nc.compile()
    return nc


def _get_program(causal: bool):
    key = ("causal" if causal else "full")
    if key not in _cache:
        _cache[key] = _build(causal)
    return _cache[key]


def _host_fallback(x, attn_mask, Wq, bq, Wk, bk, Wv, bv, Wout, bout,
                   dropout_mask):
    x64 = x.astype(np.float32)
    Q = np.einsum("btd,hdk->bhtk", x64, Wq) + bq[None, :, None, :]
    K = np.einsum("btd,hdk->bhtk", x64, Wk) + bk[None, :, None, :]
    V = np.einsum("btd,hdv->bhtv", x64, Wv) + bv[None, :, None, :]
    scores = np.einsum("bhqk,bhmk->bhqm", Q, K) * SCALE + attn_mask
    scores = scores - scores.max(-1, keepdims=True)
    e = np.exp(scores)
    attn = e / e.sum(-1, keepdims=True)
    ctx = np.einsum("bhqm,bhmv->bhqv", attn, V).reshape(B, T, H * DV)
    out = ctx @ Wout.T + bout
    return (out * dropout_mask).astype(np.float32)


def _chunked128(w):
    """[D, M] -> [128, (dc M)] with row d = dc*128 + p."""
    d, m = w.shape
    return np.ascontiguousarray(
        w.reshape(d // 128, 128, m).transpose(1, 0, 2).reshape(128, -1))


def kernel(x, attn_mask, Wq, bq, Wk, bk, Wv, bv, Wout, bout, dropout_mask):
    from concourse.bass_utils import run_bass_kernel_spmd

    x = np.ascontiguousarray(x, np.float32)
    m = np.asarray(attn_mask, np.float32).reshape(T, T)

    # causality check on the actual mask tensor
    causal = bool((np.tril(m) == 0).all() and
                  (m[np.triu_indices(T, 1)] <= -1e8).all())

    # safety: cheap bound on max |scaled score| -> exp overflow guard
    xf = x.reshape(B * T, D)
    Qa = xf @ Wq.transpose(1, 0, 2).reshape(D, H * DK)
    Ka = xf @ Wk.transpose(1, 0, 2).reshape(D, H * DK)
    Qa = Qa.reshape(B * T, H, DK) + bq[None]
    Ka = Ka.reshape(B * T, H, DK) + bk[None]
    qn = np.linalg.norm(Qa, axis=2).max(0)     # per-head max row norm
    kn = np.linalg.norm(Ka, axis=2).max(0)
    bound = float(SCALE) * float((qn * kn).max())
    if bound > 50.0:
        return _host_fallback(x, attn_mask, Wq, bq, Wk, bk, Wv, bv, Wout,
                              bout, dropout_mask)

    nc = _get_program(causal)

    xd = x.transpose(2, 0, 1).reshape(D, B * T)
    if USE_FP8:
        # [128, c2, i, B*T] with d = c2*256 + 2p + i
        xT = np.ascontiguousarray(
            xd.reshape(4, 128, 2, B * T).transpose(1, 0, 2, 3)).astype(FP8E4)
    else:
        xT = np.ascontiguousarray(
            xd.reshape(NDC, 128, B * T).transpose(1, 0, 2)
        ).astype(np.float32 if XT_F32R else BF16)
    woutT = np.asarray(Wout, np.float32).T            # [f, o]
    wout_sb = _chunked128(woutT).astype(np.float32 if WOUT_F32R else BF16)
    boutr = np.asarray(bout, np.float32).reshape(1, D)
    _w_np = np.float32 if XT_F32R else BF16

    def _wpack(w):   # [D, 128] -> [128, D] in the matching matmul layout
        if USE_FP8:
            return np.ascontiguousarray(
                w.reshape(4, 128, 2, 128).transpose(1, 0, 2, 3)
                .reshape(128, D)).astype(FP8E4)
        return _chunked128(w).astype(_w_np)
    idm = np.eye(128, dtype=np.float32) if AT_F32R else np.eye(128, dtype=np.float32).astype(BF16)
    dmask1 = np.where(np.arange(128)[None, :] < np.arange(128)[:, None],
                      MASK_NEG, np.float32(0.0)).astype(np.float32)
    dmask = np.concatenate([dmask1, dmask1], axis=1)
    maskT = None if causal else np.ascontiguousarray(m.T * np.float32(8.0))
    drop = np.asarray(dropout_mask, np.float32).astype(BF16)

    in_maps = []
    for c in range(NCORES):
        h0, h1 = HP * c, HP * c + 1
        im = {
            "xT": xT,
            "wq": _wpack(np.concatenate([Wq[h0], Wq[h1]], axis=1)),
            "wk": _wpack(np.concatenate([Wk[h0], Wk[h1]], axis=1)),
            "wv": _wpack(np.concatenate([Wv[h0], Wv[h1]], axis=1)),
            "bq": np.concatenate([bq[h0], bq[h1]]).reshape(128, 1)
                    .astype(np.float32),
            "bk": np.concatenate([bk[h0], bk[h1]]).reshape(128, 1)
                    .astype(np.float32),
            "bv": np.concatenate([bv[h0], bv[h1]]).reshape(128, 1)
                    .astype(np.float32),
            "wout": wout_sb,
            "bout": boutr,
            "onesr": np.ones((1, 128), np.float32),
            "drop": np.ascontiguousarray(drop[:, c * ROWS:(c + 1) * ROWS, :]),
            "idm": idm,
        }
        if causal:
            im["dmask"] = dmask
        else:
            im["maskT"] = maskT
        in_maps.append(im)

    res = run_bass_kernel_spmd(nc, in_maps, list(range(NCORES)))
    out = np.empty((B, T, D), np.float32)
    for c in range(NCORES):
        out[:, c * ROWS:(c + 1) * ROWS, :] = res.results[c]["out"]
    return out


# revision 39
# speedup vs baseline: 1.4077x; 1.0057x over previous
"""Trainium2 Bass kernel for nn_MultiHeadAttention_48395691492101.

Strategy: pure head-parallel sharding across 8 NeuronCores (2 heads/core).
Because the reference reshapes ctx [B,H,T,DV] -> [B,T,H*DV] WITHOUT
transposing, row-block t' in [h*128,(h+1)*128) of the reshaped tensor comes
entirely from head h.  Core c (heads 2c,2c+1) therefore owns output rows
[c*256,(c+1)*256) of every batch, and the output projection needs no
cross-core reduction at all.

Datapath (all matmuls bf16 inputs / fp32 PSUM accumulation):
- two heads fused per tile: scores/exp tiles are [128, 1024], the PV
  accumulator is [65, 1024] (64 V rows + a ones row that yields softmax
  denominators for free).
- causal: dead 128x512 blocks skipped, diagonal blocks get a triangular
  -8e9 add and column-restricted exp/PV (dead columns never touched).
- per-q-group normalization: PSUM->SBUF evict, DVE reciprocal of the sums
  row, gpsimd partition_broadcast, and the multiply folded into the Y^T
  gather copies that feed the output projection.
- flat software-pipelined emission: every chunk's PV (and, on the last
  chunk of a q-group, the accumulator evict/norm close plus boundary
  fillers - next batch's QKV t-group, V transposes one group late, the
  previous batch's output projection) is emitted only after the next
  chunk's scores, so the PE instruction stream never drains on exp or
  evict latency. Output stores are emitted one batch late so their
  semaphore waits never block a queue; a self-contained warmup matmul
  chain keeps the PE p-state ramp alive while the first x tiles stream.
"""

import sys

if "/opt/trn_rl_repo" not in sys.path:
    sys.path.insert(0, "/opt/trn_rl_repo")

import numpy as np
import ml_dtypes

BF16 = ml_dtypes.bfloat16
FP8E4 = ml_dtypes.float8_e4m3

B, T, D = 4, 2048, 1024
H, DK, DV = 16, 64, 64
SCALE = np.float32(1.0 / 8.0)
NCORES = 8
HP = H // NCORES          # heads per core = 2
ROWS = HP * (T * DV) // D  # output rows per head pair per batch = 256
NDC = D // 128            # 8 d-chunks
NTG = 4                   # t-groups of 512 for QKV
NQG = 4                   # q-groups of 512
NKC = T // 128            # 16 k-chunks
MASK_NEG = np.float32(-8.0e9)   # becomes -1e9 after *SCALE inside exp

# moving-operand dtype knobs: f32r avoids Ldweights instructions on the PE
# sequencer; bf16 halves DMA and is 1 cycle/row at any output width
XT_F32R = False
QT_F32R = False
AT_F32R = False
WOUT_F32R = False
XT_BUFS = 3
USE_FP8 = False   # fp8e4m3 + DoubleRow for QKV projections and score matmuls

_cache = {}


def _build(causal: bool, debug: bool = False):
    import concourse.tile as tile
    import concourse.mybir as mybir
    from concourse import bacc

    F32 = mybir.dt.float32
    F32R = mybir.dt.float32r
    BF = mybir.dt.bfloat16
    FP8 = mybir.dt.float8e4
    XT_DT = FP8 if USE_FP8 else (F32R if XT_F32R else BF)   # pairs with wq/wk/wv
    QT_DT = FP8 if USE_FP8 else (F32R if QT_F32R else BF)   # pairs with kt
    AT_DT = F32R if AT_F32R else BF      # pairs with vnb (and vt/id/tp chain)
    WOUT_DT = F32R if WOUT_F32R else BF  # pairs with yts
    W_DT = XT_DT
    KT_DT = QT_DT
    VNB_DT = AT_DT
    VT_DT = AT_DT
    ID_DT = AT_DT
    YT_DT = WOUT_DT
    Exp = mybir.ActivationFunctionType.Exp

    nc = bacc.Bacc("TRN2", target_bir_lowering=False, debug=False,
                   num_devices=NCORES)

    # host-prearranged layouts:
    #   fp8: xT [128, c2, i, B*T] with d = c2*256 + 2p + i, w* [128, (c2 i j)]
    #   else: xT [128, dc, B*T] with d = dc*128 + p, w* [128, (dc j)]
    if USE_FP8:
        xT_d = nc.dram_tensor("xT", [128, 4, 2, B * T], XT_DT,
                              kind="ExternalInput").ap()
    else:
        xT_d = nc.dram_tensor("xT", [128, NDC, B * T], XT_DT,
                              kind="ExternalInput").ap()
    wq_d = nc.dram_tensor("wq", [128, D], W_DT, kind="ExternalInput").ap()
    wk_d = nc.dram_tensor("wk", [128, D], W_DT, kind="ExternalInput").ap()
    wv_d = nc.dram_tensor("wv", [128, D], W_DT, kind="ExternalInput").ap()
    bq_d = nc.dram_tensor("bq", [128, 1], F32, kind="ExternalInput").ap()
    bk_d = nc.dram_tensor("bk", [128, 1], F32, kind="ExternalInput").ap()
    bv_d = nc.dram_tensor("bv", [128, 1], F32, kind="ExternalInput").ap()
    wout_d = nc.dram_tensor("wout", [128, NDC * D], WOUT_DT, kind="ExternalInput").ap()
    bout_d = nc.dram_tensor("bout", [1, D], F32R, kind="ExternalInput").ap()
    onesr_d = nc.dram_tensor("onesr", [1, 128], F32R, kind="ExternalInput").ap()
    drop_d = nc.dram_tensor("drop", [B, ROWS, D], BF, kind="ExternalInput").ap()
    id_d = nc.dram_tensor("idm", [128, 128], ID_DT, kind="ExternalInput").ap()
    if causal:
        dmask_d = nc.dram_tensor("dmask", [128, 128], AT_DT,
                                 kind="ExternalInput").ap()
    else:
        maskT_d = nc.dram_tensor("maskT", [T, T], F32, kind="ExternalInput").ap()
    out_d = nc.dram_tensor("out", [B, ROWS, D], F32, kind="ExternalOutput").ap()

    with tile.TileContext(nc) as tc:
        with tc.tile_pool(name="const", bufs=1) as cpool, \
             tc.tile_pool(name="perb", bufs=1) as perb, \
             tc.tile_pool(name="stream", bufs=3) as stream, \
             tc.tile_pool(name="psum", bufs=1, space="PSUM") as pp:

            # ---- constants ----
            # gpsimd queue: QKV weights + small tensors (needed first)
            wq_sb = cpool.tile([128, D], W_DT, tag="wq")
            wk_sb = cpool.tile([128, D], W_DT, tag="wk")
            wv_sb = cpool.tile([128, D], W_DT, tag="wv")
            nc.gpsimd.dma_start(wq_sb[:], wq_d[:])
            nc.gpsimd.dma_start(wk_sb[:], wk_d[:])
            nc.gpsimd.dma_start(wv_sb[:], wv_d[:])
            bq_sb = cpool.tile([128, 1], F32, tag="bq")
            bk_sb = cpool.tile([128, 1], F32, tag="bk")
            bv_sb = cpool.tile([128, 1], F32, tag="bv")
            nc.gpsimd.dma_start(bq_sb[:], bq_d[:])
            nc.gpsimd.dma_start(bk_sb[:], bk_d[:])
            nc.gpsimd.dma_start(bv_sb[:], bv_d[:])
            id_sb = cpool.tile([128, 128], ID_DT, tag="idm")
            nc.gpsimd.dma_start(id_sb[:], id_d[:])
            if causal:
                dmask_sb = cpool.tile([128, 128], AT_DT, tag="dmask")
                nc.gpsimd.dma_start(dmask_sb[:], dmask_d[:])
            # scalar queue: output-projection constants - loaded after the
            # prologue QKV so their transfers don't delay the first x tiles
            wout_sb = cpool.tile([128, NDC * D], WOUT_DT, tag="wout")
            bout_sb = cpool.tile([1, D], F32R, tag="bout")
            ones_row = cpool.tile([1, 128], F32R, tag="onesr")

            def emit_late_consts():
                # chunked so no single transfer hogs the DMA engines
                for cc in range(NDC):
                    nc.scalar.dma_start(wout_sb[:, cc * D:(cc + 1) * D],
                                        wout_d[:, cc * D:(cc + 1) * D])
                nc.scalar.dma_start(bout_sb[:], bout_d[:])
                nc.scalar.dma_start(ones_row[:], onesr_d[:])

            def alloc_qkv():
                qt = perb.tile([128, T], QT_DT, tag="qt", bufs=2, name="qt")
                kt = perb.tile([128, T], KT_DT, tag="kt", bufs=2, name="kt")
                vt = perb.tile([128, T], VT_DT, tag="vt", bufs=2, name="vt")
                if USE_FP8:
                    # head h on partitions [32h,32h+32); dk = 32*i + pp
                    qt8 = perb.tile([64, 2, T], QT_DT, tag="qt8", bufs=2,
                                    name="qt8")
                    kt8 = perb.tile([64, 2, T], QT_DT, tag="kt8", bufs=2,
                                    name="kt8")
                    return qt, kt, vt, qt8, kt8
                return qt, kt, vt

            def emit_remap(qkv):
                # partition fold [128,T] -> [64,2,T] via 4 SBUF->SBUF DMAs each
                qt, kt = qkv[0], qkv[1]
                qt8, kt8 = qkv[3], qkv[4]
                for pre, packed in ((qt, qt8), (kt, kt8)):
                    for h in range(2):
                        for i in range(2):
                            nc.sync.dma_start(
                                packed[32 * h:32 * h + 32, i, :],
                                pre[64 * h + 32 * i:64 * h + 32 * i + 32, :])

            def emit_qkv_tg(qkv, b, tg):
                qt, kt, vt = qkv[0], qkv[1], qkv[2]
                c0 = b * T + tg * 512
                if USE_FP8:
                    xt = stream.tile([128, 4, 2, 512], XT_DT, tag="xt",
                                     bufs=XT_BUFS, name="xt")
                    nc.sync.dma_start(xt[:], xT_d[:, :, :, c0:c0 + 512])
                else:
                    xt = stream.tile([128, NDC * 512], XT_DT, tag="xt",
                                     bufs=XT_BUFS, name="xt")
                    nc.sync.dma_start(
                        xt.rearrange("p (dc j) -> p dc j", j=512),
                        xT_d[:, :, c0:c0 + 512])
                for w_sb, bias_sb, dst in ((wq_sb, bq_sb, qt),
                                           (wk_sb, bk_sb, kt),
                                           (wv_sb, bv_sb, vt)):
                    ps = pp.tile([128, 512], F32, tag="mm", bufs=2, name="ps")
                    if USE_FP8:
                        wv8 = w_sb.rearrange("p (c2 i j) -> p c2 i j",
                                             c2=4, i=2)
                        for c2 in range(4):
                            nc.tensor.matmul(
                                ps[:], wv8[:, c2], xt[:, c2],
                                start=(c2 == 0), stop=(c2 == 3),
                                perf_mode=mybir.MatmulPerfMode.DoubleRow)
                    else:
                        for dc in range(NDC):
                            nc.tensor.matmul(
                                ps[:], w_sb[:, dc * 128:(dc + 1) * 128],
                                xt[:, dc * 512:(dc + 1) * 512],
                                start=(dc == 0), stop=(dc == NDC - 1))
                    nc.vector.tensor_scalar_add(
                        dst[:, tg * 512:(tg + 1) * 512], ps[:], bias_sb[:])

            def alloc_vnb():
                # vnb layout per k-chunk: [two heads][64 V rows + ones + pad]
                vnb = perb.tile([128, NKC * 132], VNB_DT, tag="vnb", bufs=2,
                                name="vnb")
                nc.vector.memset(
                    vnb.rearrange("p (c two w) -> p c two w", two=2, w=66)
                    [:, :, :, 64:65], 1.0)
                return vnb

            def emit_vchunks(vnb, vt, tg):
                for kc in range(4 * tg, 4 * tg + 4):
                    tp = pp.tile([128, 128], VT_DT, tag="mm", bufs=2, name="tp")
                    nc.tensor.transpose(tp[:], vt[:, kc * 128:(kc + 1) * 128],
                                        id_sb[:])
                    nc.vector.tensor_copy(
                        vnb.rearrange("p (c two w) -> p c two w", two=2, w=66)
                        [:, kc, :, 0:64],
                        tp[:].rearrange("p (two v) -> p two v", two=2))

            def emit_scores(qkv, qg, kc, live, diag):
                qt, kt = qkv[0], qkv[1]
                st = pp.tile([128, 1024], F32, tag="st", bufs=2, name="st")
                if USE_FP8:
                    qt8, kt8 = qkv[3], qkv[4]
                    for h in range(2):
                        nc.tensor.matmul(
                            st[:, h * 512 + live:(h + 1) * 512],
                            kt8[32 * h:32 * h + 32, :,
                                kc * 128:(kc + 1) * 128],
                            qt8[32 * h:32 * h + 32, :,
                                qg * 512 + live:(qg + 1) * 512],
                            start=True, stop=True,
                            perf_mode=mybir.MatmulPerfMode.DoubleRow)
                else:
                    for h in range(2):
                        nc.tensor.matmul(
                            st[:, h * 512 + live:(h + 1) * 512],
                            kt[64 * h:64 * h + 64, kc * 128:(kc + 1) * 128],
                            qt[64 * h:64 * h + 64,
                               qg * 512 + live:(qg + 1) * 512],
                            start=True, stop=(not diag))
                if diag:
                    # triangular mask added on the PE itself: accumulating
                    # matmul id^T @ dmask == dmask, so the score->exp chain
                    # never leaves the tensor engine
                    for h in range(2):
                        nc.tensor.matmul(
                            st[:, h * 512 + live:h * 512 + live + 128],
                            id_sb[:], dmask_sb[:],
                            start=False, stop=True, skip_group_check=True)
                elif not causal:
                    mt = stream.tile([128, 512], F32, tag="mt", bufs=3,
                                     name="mt")
                    nc.sync.dma_start(
                        mt[:], maskT_d[kc * 128:(kc + 1) * 128,
                                       qg * 512:(qg + 1) * 512])
                    nc.vector.tensor_add(st[:, 0:512], st[:, 0:512], mt[:])
                    nc.vector.tensor_add(st[:, 512:1024], st[:, 512:1024],
                                         mt[:])
                at = stream.tile([128, 1024], AT_DT, tag="at", bufs=4, name="at")
                nc.scalar.activation(
                    at.rearrange("p (two n) -> p two n", two=2)[:, :, live:512],
                    st.rearrange("p (two n) -> p two n", two=2)[:, :, live:512],
                    Exp, scale=float(SCALE))
                return at

            def emit_pv(cs, vnb, at, kc, live, first, last):
                for h in range(2):
                    nc.tensor.matmul(
                        cs[:, h * 512 + live:(h + 1) * 512],
                        vnb[:, kc * 132 + 66 * h:kc * 132 + 66 * h + 65],
                        at[:, h * 512 + live:(h + 1) * 512],
                        start=first, stop=last, skip_group_check=True)

            def emit_gather(ytss, ex, bcast, h, qg):
                # normalized Y^T gather for one (head, q-group) column slice
                r0, r1 = qg * 32, (qg + 1) * 32
                cvu = ex[0:64, h * T:(h + 1) * T].rearrange(
                    "p (r s2 two) -> p two s2 r", two=2, s2=8)[:, :, :, r0:r1]
                bcu = bcast[:, h * T:(h + 1) * T].rearrange(
                    "p (r s2 two) -> p two s2 r", two=2, s2=8)[:, :, :, r0:r1]
                ytv = ytss[h].rearrange("p (c r) -> p c r", r=128)
                nc.vector.tensor_mul(ytv[0:64, :, r0:r1], cvu[:, 0],
                                     bcu[:, 0])
                nc.vector.tensor_mul(ytv[64:128, :, r0:r1], cvu[:, 1],
                                     bcu[:, 1])

            def make_close(qg, cs, ex, bcast, ytss, tail=False):  # noqa: tail unused
                # evict + per-head reciprocal + broadcast + normalized gather
                def close():
                    if qg == NQG - 1:
                        # per-head evict on the scalar engine: its queue is
                        # idle at batch boundaries, so the evict dispatches
                        # immediately instead of queueing behind DVE work,
                        # and the DVE reciprocals/gathers run in parallel
                        for h in range(2):
                            s0 = h * T + qg * 512
                            nc.scalar.copy(ex[:, s0:s0 + 512],
                                           cs[:, h * 512:(h + 1) * 512])
                            rcp = stream.tile([1, 512], F32, tag="rcp",
                                              bufs=2, name="rcp")
                            nc.vector.reciprocal(rcp[:],
                                                 ex[64:65, s0:s0 + 512])
                            nc.gpsimd.partition_broadcast(
                                bcast[:, s0:s0 + 512], rcp[:])
                    else:
                        nc.vector.tensor_copy(
                            ex.rearrange("p (two t) -> p two t", two=2)
                            [:, :, qg * 512:(qg + 1) * 512],
                            cs.rearrange("p (two n) -> p two n", two=2))
                        for h in range(2):
                            s0 = h * T + qg * 512
                            rcp = stream.tile([1, 512], F32, tag="rcp",
                                              bufs=2, name="rcp")
                            nc.vector.reciprocal(rcp[:],
                                                 ex[64:65, s0:s0 + 512])
                            nc.gpsimd.partition_broadcast(
                                bcast[:, s0:s0 + 512], rcp[:])
                        for h in range(2):
                            emit_gather(ytss, ex, bcast, h, qg)
                return close

            pend_store = {0: None, 1: None}

            def flush_store(h):
                if pend_store[h] is not None:
                    dst, ost = pend_store[h]
                    nc.sync.dma_start(dst, ost[:])
                    pend_store[h] = None

            def emit_phase5(b, h, ex, bcast, dt2s, ytss, final=False):
                dt2 = dt2s[h]
                yts = ytss[h]
                flush_store(h)
                emit_gather(ytss, ex, bcast, h, NQG - 1)
                ost = stream.tile([128, D], F32, tag="ost", bufs=2, name="ost")
                for og in range(2):
                    po = pp.tile([128, 512], F32, tag="mm", bufs=2, name="po")
                    for cc in range(NDC):
                        nc.tensor.matmul(
                            po[:], yts[:, cc * 128:(cc + 1) * 128],
                            wout_sb[:, cc * D + og * 512:cc * D + og * 512 + 512],
                            start=(cc == 0), stop=False)
                    nc.tensor.matmul(po[:], ones_row[:],
                                     bout_sb[0:1, og * 512:(og + 1) * 512],
                                     start=False, stop=True)
                    nc.vector.tensor_mul(
                        ost[:, og * 512:(og + 1) * 512], po[:],
                        dt2[:, og * 512:(og + 1) * 512])
                    if final:
                        nc.sync.dma_start(
                            out_d[b, h * 128:(h + 1) * 128,
                                  og * 512:(og + 1) * 512],
                            ost[:, og * 512:(og + 1) * 512])
                if not final:
                    pend_store[h] = (out_d[b, h * 128:(h + 1) * 128, :], ost)

            # PE p-state warmup: self-contained matmul chain (no DMA deps)
            # keeps the clock ramp alive while the first x tiles stream in
            warm_sb = cpool.tile([128, 512], BF, tag="warm")
            nc.vector.memset(warm_sb[:], 0.0)

            def emit_warmup(n):
                wp = pp.tile([128, 512], F32, tag="mm", bufs=2, name="wp")
                for i in range(n):
                    nc.tensor.matmul(wp[:], warm_sb[:, 0:128], warm_sb[:],
                                     start=True, stop=True)

            # ============ flat software-pipelined schedule ============
            # Each chunk's PV (and, on the last chunk of a q-group, the
            # evict/norm close plus the boundary fillers: next batch's QKV,
            # V-transposes, previous batch's output projection) is emitted
            # only AFTER the next chunk's scores, so the PE stream never
            # drains waiting on exp or the accumulator evict.
            cur = alloc_qkv()
            cur_vnb = alloc_vnb()
            emit_warmup(14)
            if USE_FP8:
                # fp8 remap needs the full qt/kt before attention
                for tg in range(NTG):
                    emit_qkv_tg(cur, 0, tg)
                    if tg > 0:
                        emit_vchunks(cur_vnb, cur[2], tg - 1)
                emit_remap(cur)
                emit_vchunks(cur_vnb, cur[2], NTG - 1)
            else:
                # attention q-group qg only needs t-groups <= qg of Q/K and
                # V chunks <= qg, so b0's tg2/tg3 become boundary fillers
                # inside its own early attention instead of serial prologue
                emit_qkv_tg(cur, 0, 0)
                emit_qkv_tg(cur, 0, 1)
                emit_vchunks(cur_vnb, cur[2], 0)

            carry = {"v": None}  # pending PV (+close/fillers) of prev chunk
            pend_vchunks = []    # deferred tg3 V-transposes of the next batch

            def flush_carry():
                c = carry["v"]
                if c is None:
                    return
                emit_pv(*c["pv"])
                if c["close"] is not None:
                    c["close"]()
                for f in c["fillers"]:
                    f()
                carry["v"] = None

            prev = None   # (b-1's ex, bcast, dt2s) - phase5 runs inside attn(b)
            for b in range(B):
                ex = perb.tile([65, 2 * T], F32, tag="ex", bufs=2, name="ex")
                bcast = perb.tile([64, 2 * T], F32, tag="bc", bufs=2,
                                  name="bcast")
                ytss = [stream.tile([128, NDC * 128], YT_DT, tag="yt",
                                    bufs=4, name="yts") for _ in range(HP)]
                if b + 1 < B:
                    nxt = alloc_qkv()
                    nxt_vnb = alloc_vnb()
                dt2s = []
                for qg in range(NQG):
                    kcmax = 4 * qg + 4 if causal else NKC
                    cs = pp.tile([65, 1024], F32, tag="cs", bufs=1, name="cs")
                    fillers = []
                    if qg == 0 and pend_vchunks:
                        fillers.extend(pend_vchunks)
                        pend_vchunks.clear()
                    if b == 0 and not USE_FP8:
                        if qg < 2:
                            fillers.append(
                                lambda cc=cur, tg=qg + 2:
                                    emit_qkv_tg(cc, 0, tg))
                            fillers.append(
                                lambda vb=cur_vnb, vt=cur[2], tg=qg + 1:
                                    emit_vchunks(vb, vt, tg))
                        elif qg == 2:
                            fillers.append(
                                lambda vb=cur_vnb, vt=cur[2]:
                                    emit_vchunks(vb, vt, 3))
                    if b == 0 and qg == 1:
                        fillers.append(emit_late_consts)
                    if b + 1 < B:
                        fillers.append(
                            lambda nx=nxt, bb=b + 1, tg=qg:
                                emit_qkv_tg(nx, bb, tg))
                        if qg > 0:
                            fillers.append(
                                lambda vb=nxt_vnb, vt=nxt[2], tg=qg - 1:
                                    emit_vchunks(vb, vt, tg))
                    p5qg = qg - 1
                    if prev is not None and 0 <= p5qg < HP:
                        fillers.append(
                            lambda bb=b - 1, h=p5qg, pv=prev:
                                emit_phase5(bb, h, *pv))
                    if qg == NQG - 1:
                        def _dt(b=b, dt2s=dt2s):
                            for h in range(HP):
                                dt2 = stream.tile([128, D], BF, tag="dt",
                                                  bufs=2, name="dt2")
                                nc.sync.dma_start(
                                    dt2[:],
                                    drop_d[b, h * 128:(h + 1) * 128, :])
                                dt2s.append(dt2)
                        fillers.append(_dt)
                        if USE_FP8 and b + 1 < B:
                            fillers.append(lambda nx=nxt: emit_remap(nx))
                        if b + 1 < B:
                            pend_vchunks.append(
                                lambda vb=nxt_vnb, vt=nxt[2]:
                                    emit_vchunks(vb, vt, NTG - 1))
                    close = make_close(qg, cs, ex, bcast, ytss,
                                       tail=(b == B - 1))
                    for kc in range(kcmax):
                        o = kc - 4 * qg
                        diag = causal and o >= 0
                        live = o * 128 if diag else 0
                        at = emit_scores(cur, qg, kc, live, diag)
                        flush_carry()
                        last = (kc == kcmax - 1)
                        carry["v"] = {
                            "pv": (cs, cur_vnb, at, kc, live, kc == 0, last),
                            "close": close if last else None,
                            "fillers": fillers if last else [],
                        }
                prev = (ex, bcast, dt2s, ytss)
                if b + 1 < B:
                    cur, cur_vnb = nxt, nxt_vnb
            flush_carry()
            for h in range(HP):
                emit_phase5(B - 1, h, *prev, final=True)
            flush_store(0)
            flush_store(1)

    nc.compile()
    return nc


def _get_program(causal: bool):
    key = ("causal" if causal else "full")
    if key not in _cache:
        _cache[key] = _build(causal)
    return _cache[key]


def _host_fallback(x, attn_mask, Wq, bq, Wk, bk, Wv, bv, Wout, bout,
                   dropout_mask):
    x64 = x.astype(np.float32)
    Q = np.einsum("btd,hdk->bhtk", x64, Wq) + bq[None, :, None, :]
    K = np.einsum("btd,hdk->bhtk", x64, Wk) + bk[None, :, None, :]
    V = np.einsum("btd,hdv->bhtv", x64, Wv) + bv[None, :, None, :]
    scores = np.einsum("bhqk,bhmk->bhqm", Q, K) * SCALE + attn_mask
    scores = scores - scores.max(-1, keepdims=True)
    e = np.exp(scores)
    attn = e / e.sum(-1, keepdims=True)
    ctx = np.einsum("bhqm,bhmv->bhqv", attn, V).reshape(B, T, H * DV)
    out = ctx @ Wout.T + bout
    return (out * dropout_mask).astype(np.float32)


def _chunked128(w):
    """[D, M] -> [128, (dc M)] with row d = dc*128 + p."""
    d, m = w.shape
    return np.ascontiguousarray(
        w.reshape(d // 128, 128, m).transpose(1, 0, 2).reshape(128, -1))


def kernel(x, attn_mask, Wq, bq, Wk, bk, Wv, bv, Wout, bout, dropout_mask):
    from concourse.bass_utils import run_bass_kernel_spmd

    x = np.ascontiguousarray(x, np.float32)
    m = np.asarray(attn_mask, np.float32).reshape(T, T)

    # causality check on the actual mask tensor
    causal = bool((np.tril(m) == 0).all() and
                  (m[np.triu_indices(T, 1)] <= -1e8).all())

    # safety: cheap bound on max |scaled score| -> exp overflow guard
    xf = x.reshape(B * T, D)
    Qa = xf @ Wq.transpose(1, 0, 2).reshape(D, H * DK)
    Ka = xf @ Wk.transpose(1, 0, 2).reshape(D, H * DK)
    Qa = Qa.reshape(B * T, H, DK) + bq[None]
    Ka = Ka.reshape(B * T, H, DK) + bk[None]
    qn = np.linalg.norm(Qa, axis=2).max(0)     # per-head max row norm
    kn = np.linalg.norm(Ka, axis=2).max(0)
    bound = float(SCALE) * float((qn * kn).max())
    if bound > 50.0:
        return _host_fallback(x, attn_mask, Wq, bq, Wk, bk, Wv, bv, Wout,
                              bout, dropout_mask)

    nc = _get_program(causal)

    xd = x.transpose(2, 0, 1).reshape(D, B * T)
    if USE_FP8:
        # [128, c2, i, B*T] with d = c2*256 + 2p + i
        xT = np.ascontiguousarray(
            xd.reshape(4, 128, 2, B * T).transpose(1, 0, 2, 3)).astype(FP8E4)
    else:
        xT = np.ascontiguousarray(
            xd.reshape(NDC, 128, B * T).transpose(1, 0, 2)
        ).astype(np.float32 if XT_F32R else BF16)
    woutT = np.asarray(Wout, np.float32).T            # [f, o]
    wout_sb = _chunked128(woutT).astype(np.float32 if WOUT_F32R else BF16)
    boutr = np.asarray(bout, np.float32).reshape(1, D)
    _w_np = np.float32 if XT_F32R else BF16

    def _wpack(w):   # [D, 128] -> [128, D] in the matching matmul layout
        if USE_FP8:
            return np.ascontiguousarray(
                w.reshape(4, 128, 2, 128).transpose(1, 0, 2, 3)
                .reshape(128, D)).astype(FP8E4)
        return _chunked128(w).astype(_w_np)
    idm = np.eye(128, dtype=np.float32) if AT_F32R else np.eye(128, dtype=np.float32).astype(BF16)
    dmask = np.where(np.arange(128)[None, :] < np.arange(128)[:, None],
                     MASK_NEG, np.float32(0.0)).astype(
                         np.float32 if AT_F32R else BF16)
    maskT = None if causal else np.ascontiguousarray(m.T * np.float32(8.0))
    drop = np.asarray(dropout_mask, np.float32).astype(BF16)

    in_maps = []
    for c in range(NCORES):
        h0, h1 = HP * c, HP * c + 1
        im = {
            "xT": xT,
            "wq": _wpack(np.concatenate([Wq[h0], Wq[h1]], axis=1)),
            "wk": _wpack(np.concatenate([Wk[h0], Wk[h1]], axis=1)),
            "wv": _wpack(np.concatenate([Wv[h0], Wv[h1]], axis=1)),
            "bq": np.concatenate([bq[h0], bq[h1]]).reshape(128, 1)
                    .astype(np.float32),
            "bk": np.concatenate([bk[h0], bk[h1]]).reshape(128, 1)
                    .astype(np.float32),
            "bv": np.concatenate([bv[h0], bv[h1]]).reshape(128, 1)
                    .astype(np.float32),
            "wout": wout_sb,
            "bout": boutr,
            "onesr": np.ones((1, 128), np.float32),
            "drop": np.ascontiguousarray(drop[:, c * ROWS:(c + 1) * ROWS, :]),
            "idm": idm,
        }
        if causal:
            im["dmask"] = dmask
        else:
            im["maskT"] = maskT
        in_maps.append(im)

    res = run_bass_kernel_spmd(nc, in_maps, list(range(NCORES)))
    out = np.empty((B, T, D), np.float32)
    for c in range(NCORES):
        out[:, c * ROWS:(c + 1) * ROWS, :] = res.results[c]["out"]
    return out
